# revision 40
# baseline (speedup 1.0000x reference)
"""TRN2 Bass kernel for nn_AttentionModel_46823733461774.

Gemma3n-style attention block: qkv projection, q/k/v RMS-norm, RoPE on q/k,
GQA causal attention (no scaling; q_norm replaces 1/sqrt(d)), output proj.

Shapes (hardcoded): B=2, S=2048, D=2048, H=8, KV=2, DH=256.

Sharding over 8 cores: core c -> batch b=c//4, q-heads {2j, 2j+1} (j=c%4),
kv-head j//2.  Each core computes the projections for its batch/heads
(token-major), norms+RoPE, causal attention for its 2 heads, and a partial
output projection attn_heads @ wo_slice^T.  Host sums the 4 partials per
batch.  cos/sin replicated.

All matmuls in fp16 (same PE throughput as bf16, 8x the mantissa accuracy);
softmax statistics and accumulations in fp32.

Single merged pipeline: projection tiles and attention tiles are emitted
interleaved (attention for token-tile i follows projection of tile i+3), so
every engine keeps independent work during the softmax dependency chains.
Scores are computed in 512-column PSUM chunks from a 4-bank ring; the causal
mask is added to the diagonal chunk on DVE; row maxes are reduced per-chunk
on DVE as each chunk's matmuls finish; exp runs per-chunk on ACT; prob
transposes + PV accumulate chunk-by-chunk; 256-wide output-proj matmul
chains are used as fine-grained PE filler inside the softmax shadows.

Input DMA is pipelined in consumption order on the SP queue (per-queue DMA
transfers serialize): first xT chunk in dt-halves + wq quarters so the
tile-0 projection starts ~10us in, then wkv, rope, remaining xT chunks, wo.

Further structure: x^T is SBUF-resident; q^T / attn^T live in small ring
buffers; rms rsqrt is computed as exp(-0.5*ln(x)) so every ACT function
(square/ln/exp/copy) lives in one activation table (no 1.3us table
reloads); when the norm weights are all-ones (the reference setup), a
single packed half-table [cos|-sin|+sin] serves q and k rope (1.5MB instead
of 4MB of DMA + SBUF), with a full-table fallback otherwise.
"""

import os
from collections import deque

import numpy as np
import ml_dtypes

import concourse.bass as bass
import concourse.mybir as mybir
import concourse.tile as tile
from concourse import bacc
from concourse import bass_utils

B, S, D = 2, 2048, 2048
H, KV, DH = 8, 2, 256
EPS = 1e-6
NEG = -30000.0   # additive causal mask (fp16-representable; exp() -> 0)
P = 128
TT = S // P      # 16 token tiles
DT = D // P      # 16 contraction tiles
NH = 2           # heads per core
KC = 512         # key chunk (scores free dim; one PSUM bank)

# matmul dtype mode: "f16" | "bf16" | "f32"
MODE = os.environ.get("KERNEL_MODE", "f16")
# repeat the body N times inside the NEFF (for wall-clock HW timing)
ITERS = int(os.environ.get("KERNEL_ITERS", "1"))

_cache = {}


def _np_md():
    if MODE == "bf16":
        return ml_dtypes.bfloat16
    if MODE == "f16":
        return np.float16
    return np.float32


def _bir_md():
    if MODE == "bf16":
        return mybir.dt.bfloat16
    if MODE == "f16":
        return mybir.dt.float16
    return mybir.dt.float32


def _build_program(compact_rope=True):
    f32 = mybir.dt.float32
    md = _bir_md()
    Alu = mybir.AluOpType
    Act = mybir.ActivationFunctionType
    X = mybir.AxisListType.X
    XY = mybir.AxisListType.XY

    nc = bacc.Bacc("TRN2", target_bir_lowering=False, debug=False, num_devices=8)

    # fp16 buffers hang at the PJRT/axon boundary -> declare 2-byte inputs
    # as uint16 and bitcast to the matmul dtype on the DRAM APs.
    io2 = mybir.dt.uint16 if mybir.dt.size(md) == 2 else md
    def _in2(name, shape):
        ap = nc.dram_tensor(name, shape, io2, kind="ExternalInput").ap()
        return ap.bitcast(md) if io2 != md else ap
    xT_d = _in2("xT", [D, S])
    wqT_d = _in2("wqT", [D, NH * DH])
    wkvT_d = _in2("wkvT", [D, 2 * DH])
    woT2_d = _in2("woT2", [NH * DH, D])
    if compact_rope:
        # norm weights are all-ones and cos/sin halves are identical, so a
        # single packed table [S, 3*hd] = [cos_half | -sin_half | +sin_half]
        # serves q and k (1.5MB instead of 4MB of DMA + SBUF).
        rope3_d = _in2("rope3", [S, 3 * (DH // 2)])
    else:
        # rope tables with the norm weight and rotate-half signs folded in:
        # cw = cos*w ; sw[d<hd] = -sin[d]*w[d+hd], sw[d>=hd] = sin[d]*w[d-hd]
        cqw_d = _in2("cqw", [S, DH])
        sqw_d = _in2("sqw", [S, DH])
        ckw_d = _in2("ckw", [S, DH])
        skw_d = _in2("skw", [S, DH])
    trimaskf_d = nc.dram_tensor("trimaskf", [P, P], f32,
                                kind="ExternalInput").ap()
    ident_d = _in2("ident", [P, P])
    # output partials in bf16 (halves the out DMA; host sums in fp32).
    # 2-byte IO declared as uint16 like the inputs (PJRT boundary quirk).
    if io2 == md:  # f32 mode
        out_d = nc.dram_tensor("out", [S, D], f32, kind="ExternalOutput").ap()
        out_md = f32
    else:
        out_d = nc.dram_tensor("out", [S, D], mybir.dt.uint16,
                               kind="ExternalOutput").ap().bitcast(
                                   mybir.dt.bfloat16)
        out_md = mybir.dt.bfloat16

    with tile.TileContext(nc) as tc:
        with (
            tc.tile_pool(name="const", bufs=1) as cpool,
            tc.tile_pool(name="resid", bufs=1) as rpool,
            tc.tile_pool(name="pbuf", bufs=8) as ppool,
            tc.tile_pool(name="ptbuf", bufs=6) as ptpool,
            tc.tile_pool(name="tmp", bufs=10) as tpool,
            tc.tile_pool(name="stat", bufs=12) as spool,
            tc.tile_pool(name="obuf", bufs=3) as opool,
            tc.tile_pool(name="psum", bufs=1, space="PSUM") as psum,
        ):
            # ---- SBUF tiles for constants / weights / x ----
            wq_sb = cpool.tile([P, DT, NH * DH], md, tag="wq")
            wkv_sb = cpool.tile([P, DT, 2 * DH], md, tag="wkv")
            wo_sb = cpool.tile([P, NH * DH // P, D], md, tag="wo")
            xT_sb = cpool.tile([P, DT, S], md, tag="xT")
            hd = DH // 2
            if compact_rope:
                rope_sb = cpool.tile([P, TT, 3 * hd], md, tag="rope3")
            else:
                cqw_sb = cpool.tile([P, TT, DH], md, tag="cqw")
                sqw_sb = cpool.tile([P, TT, DH], md, tag="sqw")
                ckw_sb = cpool.tile([P, TT, DH], md, tag="ckw")
                skw_sb = cpool.tile([P, TT, DH], md, tag="skw")
            trif_sb = cpool.tile([P, P], f32, tag="trif")
            ident = cpool.tile([P, P], md, tag="ident")
            eps_sb = cpool.tile([P, 1], f32, tag="eps")

            # ---- pipelined input DMA, in consumption order ----
            # xT in 256-token chunks (512B contiguous runs, full DMA speed);
            # weights in halves so the first proj matmuls can start early;
            # rope tables chunked alongside the x tiles they feed; wo last
            # (first consumer is the tile-0 output chain, ~4 tiles in).
            XC = 256
            NXC = S // XC

            def load_xt(ci):
                nc.sync.dma_start(
                    xT_sb[:, :, ci * XC:(ci + 1) * XC],
                    xT_d[:, ci * XC:(ci + 1) * XC].rearrange(
                        "(dt p) t -> p dt t", p=P))

            def load_rope(ci):
                if compact_rope:
                    nc.sync.dma_start(
                        rope_sb[:, 2 * ci:2 * ci + 2, :],
                        rope3_d[ci * XC:(ci + 1) * XC, :].rearrange(
                            "(tt p) d1 -> p tt d1", p=P))
                else:
                    for sb, dr in ((cqw_sb, cqw_d), (sqw_sb, sqw_d),
                                   (ckw_sb, ckw_d), (skw_sb, skw_d)):
                        nc.sync.dma_start(
                            sb[:, 2 * ci:2 * ci + 2, :],
                            dr[ci * XC:(ci + 1) * XC, :].rearrange(
                                "(tt p) d1 -> p tt d1", p=P))

            # first xT chunk in dt-halves interleaved with wq quarters, so
            # the tile-0 q-projection chain starts as soon as the first
            # 0.5MB pieces land instead of after 2.5MB
            QDT = DT // 4
            def load_wq_q(qi):
                nc.sync.dma_start(
                    wq_sb[:, qi * QDT:(qi + 1) * QDT, :],
                    wqT_d[qi * D // 4:(qi + 1) * D // 4, :].rearrange(
                        "(dt p) e -> p dt e", p=P))

            nc.sync.dma_start(
                xT_sb[:, 0:DT // 2, 0:XC],
                xT_d[0:D // 2, 0:XC].rearrange("(dt p) t -> p dt t", p=P))
            load_wq_q(0)
            load_wq_q(1)
            nc.sync.dma_start(
                xT_sb[:, DT // 2:DT, 0:XC],
                xT_d[D // 2:D, 0:XC].rearrange("(dt p) t -> p dt t", p=P))
            load_wq_q(2)
            load_wq_q(3)
            HDT = DT // 2
            for h2 in range(2):
                nc.sync.dma_start(
                    wkv_sb[:, h2 * HDT:(h2 + 1) * HDT, :],
                    wkvT_d[h2 * D // 2:(h2 + 1) * D // 2, :].rearrange(
                        "(dt p) e -> p dt e", p=P))
            nc.sync.dma_start(trif_sb[:], trimaskf_d)
            nc.sync.dma_start(ident[:], ident_d)
            nc.gpsimd.memset(eps_sb[:], EPS)
            load_rope(0)
            for ci in range(1, 4):
                load_xt(ci)
                load_rope(ci)
            nc.sync.dma_start(
                wo_sb[:], woT2_d.rearrange("(et p) d1 -> p et d1", p=P))
            for ci in range(4, NXC):
                load_xt(ci)
                load_rope(ci)

            # ---- persistent activations ----
            kT_sb = rpool.tile([P, 2, S], md, tag="kT")
            v_sb = rpool.tile([P, TT, DH], md, tag="v")        # token-major

            env = dict(
                f32=f32, md=md, Alu=Alu, Act=Act, X=X, XY=XY,
                wq_sb=wq_sb, wkv_sb=wkv_sb, wo_sb=wo_sb, trif_sb=trif_sb,
                ident=ident, eps_sb=eps_sb, kT_sb=kT_sb,
                v_sb=v_sb, xT_sb=xT_sb, out_d=out_d, out_md=out_md,
                ppool=ppool, ptpool=ptpool, tpool=tpool,
                spool=spool, opool=opool, psum=psum,
                qT_tiles={}, aT_tiles={},
            )
            if compact_rope:
                env["rope_sb"] = rope_sb
            else:
                env.update(cqw_sb=cqw_sb, sqw_sb=sqw_sb,
                           ckw_sb=ckw_sb, skw_sb=skw_sb)
            env["compact_rope"] = compact_rope
            import contextlib
            unroll = int(os.environ.get("KERNEL_UNROLL", "1"))
            loop_ctx = (tc.For_i(0, ITERS, 1) if ITERS > 1
                        else contextlib.nullcontext())
            with loop_ctx:
                for _ in range(unroll):
                    _emit_body(nc, tc, env)

    # Activation-table pre-placement: the stock pass greedily maps each
    # activation to the FIRST act_info.json set containing its function,
    # which ping-pongs between the exp table and the ln table (1.3us reload
    # each).  Pre-place loads with our functions masked out of every set
    # before natural_log_exp_and_others, so everything first-matches that
    # one set (it contains exp+ln+square+copy+identity) and a single load
    # suffices.  Set ids keep their original act_info.json indices.
    from concourse.hw_specs import get_activation_tables
    import bass_rust as _br
    Act = mybir.ActivationFunctionType
    tables = list(get_activation_tables(nc.m.arch).items())
    target = next(idx for idx, (n, s) in enumerate(tables)
                  if n == "natural_log_exp_and_others")
    ours = {Act.Square, Act.Ln, Act.Exp, Act.Copy, Act.Identity}
    tables = [(n, (s - ours) if idx < target else s)
              for idx, (n, s) in enumerate(tables)]
    _br.insert_act_table_loads(nc, tables)

    nc.compile()
    return nc


# PSUM bank budget (8 banks): score-chunk ring "s" x4, transpose landing
# "t" x2, PV-accumulator / out-proj chain ring "ao" x2.
S_BUFS = 4
T_BUFS = 2
AO_BUFS = 2
PRE = 3  # attention for tile i is emitted after projection of tile i+PRE


def _emit_body(nc, tc, env):
    fillers = deque()

    def pop_fill(n):
        for _ in range(min(n, len(fillers))):
            fillers.popleft()()

    for t in range(TT + PRE):
        i = t - PRE
        pj = _emit_proj_q(nc, tc, env, t) if t < TT else None
        if pj is not None:
            _emit_proj_kv(nc, tc, env, t, pj)
        if 0 <= i < TT:
            pk = _emit_attn_scores(nc, tc, env, i, 0)
            pop_fill(4)
            _emit_attn_rest(nc, tc, env, i, 0, pk)
            if pj is not None:
                _emit_proj_rope(nc, tc, env, t, pj, part=0)
            pk = _emit_attn_scores(nc, tc, env, i, 1)
            pop_fill(4)
            _emit_attn_rest(nc, tc, env, i, 1, pk)
            if pj is not None:
                _emit_proj_rope(nc, tc, env, t, pj, part=1)
            for dc in range(D // 256):
                fillers.append(
                    lambda i=i, dc=dc: _emit_op_chain(nc, tc, env, i, dc))
        elif pj is not None:
            _emit_proj_kv(nc, tc, env, t, pj)
            _emit_proj_rope(nc, tc, env, t, pj, part=0)
            _emit_proj_rope(nc, tc, env, t, pj, part=1)
    pop_fill(len(fillers))


def _stat_pair(nc, env, srcs, pair):
    """rr = exp(-0.5*ln(mean(x^2)+eps)) for a pair of DH-wide sources.
    Ln+Exp share an ACT function table with Square/Copy: no table reloads."""
    f32, Act = env["f32"], env["Act"]
    tpool, spool = env["tpool"], env["spool"]
    ss2 = spool.tile([P, 2], f32, tag=f"ss{pair}", name="ss2")
    for j in (0, 1):
        sq = tpool.tile([P, DH], f32, tag="sq", bufs=2)
        nc.scalar.activation(sq[:], srcs[j], Act.Square,
                             accum_out=ss2[:, j:j + 1])
    lg2 = spool.tile([P, 2], f32, tag=f"lg{pair}", name="lg2")
    nc.scalar.activation(lg2[:], ss2[:], Act.Ln,
                         bias=env["eps_sb"][:], scale=1.0 / DH)
    rr2 = spool.tile([P, 2], f32, tag=f"rr{pair}", name="rr2")
    nc.scalar.activation(rr2[:], lg2[:], Act.Exp, scale=-0.5)
    return rr2


def _emit_proj_q(nc, tc, env, tt):
    """q projection matmuls for token tile tt + PSUM->SBUF evacuation + rms
    statistics.  The kv half is emitted separately (inside the h0 softmax
    shadow) via _emit_proj_kv."""
    f32, md = env["f32"], env["md"]
    xT_sb = env["xT_sb"]
    tpool, psum = env["tpool"], env["psum"]

    q_ps = psum.tile([P, NH * DH], f32, tag="s", bufs=S_BUFS, name="q_ps")
    for d in range(DT):
        nc.tensor.matmul(q_ps[:], xT_sb[:, d, tt * P:(tt + 1) * P],
                         env["wq_sb"][:, d, :],
                         start=(d == 0), stop=(d == DT - 1))
    qsb = tpool.tile([P, NH * DH], md, tag="qsb", bufs=3)
    nc.scalar.copy(qsb[:], q_ps[:])
    rr_q = _stat_pair(nc, env, [qsb[:, 0:DH], qsb[:, DH:2 * DH]], 0)
    return dict(qsb=qsb, rr_q=rr_q)


def _emit_proj_kv(nc, tc, env, tt, pj):
    f32, md = env["f32"], env["md"]
    xT_sb = env["xT_sb"]
    tpool, psum = env["tpool"], env["psum"]

    kv_ps = psum.tile([P, 2 * DH], f32, tag="s", bufs=S_BUFS, name="kv_ps")
    for d in range(DT):
        nc.tensor.matmul(kv_ps[:], xT_sb[:, d, tt * P:(tt + 1) * P],
                         env["wkv_sb"][:, d, :],
                         start=(d == 0), stop=(d == DT - 1))
    kvsb = tpool.tile([P, 2 * DH], md, tag="kvsb", bufs=3)
    nc.scalar.copy(kvsb[:], kv_ps[:])
    rr_kv = _stat_pair(nc, env, [kvsb[:, 0:DH], kvsb[:, DH:2 * DH]], 1)
    pj["kvsb"] = kvsb
    pj["rr_kv"] = rr_kv


def _emit_proj_rope(nc, tc, env, tt, pj, part):
    """part 0: q heads norm+rope; part 1: k norm+rope and v norm.  rope+norm
    in ~5 DVE ops per head:
      u  = (src * rr) * c           (c = cos table)
      v  = (rot_half(src) * rr) * s   (signs folded into s; 2 half-ops)
      qr = u + v
    Head-major transposes run on the DMA engines (XBAR 128-block transpose),
    keeping the PE free."""
    f32, md = env["f32"], env["md"]
    Alu = env["Alu"]
    kT_sb, v_sb = env["kT_sb"], env["v_sb"]
    tpool = env["tpool"]

    hd = DH // 2
    compact = env["compact_rope"]
    whichs = (0, 1) if part == 0 else (2,)
    psum = env["psum"]
    ident = env["ident"]
    qr2 = tpool.tile([P, len(whichs) * DH], md,
                     tag="qr2" if part == 0 else "kr2", bufs=3)
    for slot, which in enumerate(whichs):
        if which < NH:
            src = pj["qsb"][:, which * DH:(which + 1) * DH]
            rr = pj["rr_q"][:, which:which + 1]
        else:
            src = pj["kvsb"][:, 0:DH]
            rr = pj["rr_kv"][:, 0:1]
        if compact:
            ch = env["rope_sb"][:, tt, 0:hd]
            sn = env["rope_sb"][:, tt, hd:2 * hd]
            sp = env["rope_sb"][:, tt, 2 * hd:3 * hd]
        else:
            if which < NH:
                cw = env["cqw_sb"][:, tt, :]
                sw = env["sqw_sb"][:, tt, :]
            else:
                cw = env["ckw_sb"][:, tt, :]
                sw = env["skw_sb"][:, tt, :]
        u = tpool.tile([P, DH], md, tag="qa", bufs=3)
        if compact:
            nc.vector.scalar_tensor_tensor(u[:, 0:hd], src[:, 0:hd], rr, ch,
                                           op0=Alu.mult, op1=Alu.mult)
            nc.vector.scalar_tensor_tensor(u[:, hd:DH], src[:, hd:DH], rr, ch,
                                           op0=Alu.mult, op1=Alu.mult)
        else:
            nc.vector.scalar_tensor_tensor(u[:], src, rr, cw,
                                           op0=Alu.mult, op1=Alu.mult)
        v = tpool.tile([P, DH], md, tag="t1", bufs=3)
        nc.vector.scalar_tensor_tensor(v[:, 0:hd], src[:, hd:DH], rr,
                                       sn if compact else sw[:, 0:hd],
                                       op0=Alu.mult, op1=Alu.mult)
        nc.vector.scalar_tensor_tensor(v[:, hd:DH], src[:, 0:hd], rr,
                                       sp if compact else sw[:, hd:DH],
                                       op0=Alu.mult, op1=Alu.mult)
        nc.vector.tensor_add(qr2[:, slot * DH:(slot + 1) * DH], u[:], v[:])
    # PE transpose into head-major layout, then one batched DVE copy
    nblk = 2 * len(whichs)
    tp_ps = psum.tile([P, 512], md, tag="t", bufs=T_BUFS, name="tp_ps")
    for blk in range(nblk):
        nc.tensor.transpose(tp_ps[:, blk * P:(blk + 1) * P],
                            qr2[:, blk * P:(blk + 1) * P], ident[:])
    if part == 0:
        qT_t = tpool.tile([P, NH * 2, P], md, tag="qTt", bufs=5, name="qT_t")
        nc.vector.tensor_copy(
            qT_t[:], tp_ps[:].rearrange("p (b q1) -> p b q1", b=4))
        env["qT_tiles"][tt] = qT_t
    else:
        nc.vector.tensor_copy(
            kT_sb[:, :, tt * P:(tt + 1) * P],
            tp_ps[:, 0:2 * P].rearrange("p (b q1) -> p b q1", b=2))
        # ---- v: rms-norm only, stays token-major (fp16 SBUF -> 4x mode)
        nc.vector.tensor_scalar_mul(v_sb[:, tt, :], pj["kvsb"][:, DH:2 * DH],
                                    pj["rr_kv"][:, 1:2])


def _emit_attn_scores(nc, tc, env, i, h):
    """Scores in 512-col PSUM chunks + per-chunk max + exp.  Returns the
    packet (pchunks, zs, W) consumed by _emit_attn_rest."""
    f32, md = env["f32"], env["md"]
    Alu, Act, X = env["Alu"], env["Act"], env["X"]
    kT_sb = env["kT_sb"]
    trif_sb = env["trif_sb"]
    ppool, spool, psum = env["ppool"], env["spool"], env["psum"]
    qT_t = env["qT_tiles"][i]

    nlive = i + 1
    W = (nlive * P + KC - 1) // KC
    mxs = spool.tile([P, 5], f32, tag="mxs", name="mxs")
    schunks = []
    for c in range(W):
        k0 = c * KC
        k1 = min((c + 1) * KC, nlive * P)
        wc = k1 - k0
        s = psum.tile([P, KC], f32, tag="s", bufs=S_BUFS, name="s")
        for dh in (0, 1):
            nc.tensor.matmul(
                s[:, 0:wc], qT_t[:, h * 2 + dh, :],
                kT_sb[:, dh, k0:k1],
                start=(dh == 0), stop=(dh == 1))
        if c == W - 1:
            # additive causal mask on the diagonal 128x128 block (DVE)
            nc.vector.tensor_tensor(s[:, wc - P:wc], s[:, wc - P:wc],
                                    trif_sb[:], op=Alu.add)
        nc.vector.tensor_reduce(mxs[:, c:c + 1], s[:, 0:wc], axis=X,
                                op=Alu.max, negate=True)
        schunks.append((s, wc))
    negm = spool.tile([P, 1], f32, tag="negm", name="negm")
    nc.vector.tensor_reduce(negm[:], mxs[:, 0:W], axis=X, op=Alu.min)

    zs = spool.tile([P, 4], f32, tag="zs", name="zs")
    pchunks = []
    for c, (s, wc) in enumerate(schunks):
        p = ppool.tile([P, KC], md, tag="p")
        nc.scalar.activation(p[:, 0:wc], s[:, 0:wc], Act.Exp,
                             bias=negm[:], accum_out=zs[:, c:c + 1])
        pchunks.append((p, wc))
    return (pchunks, zs, W)


def _emit_attn_rest(nc, tc, env, i, h, pk):
    f32, md = env["f32"], env["md"]
    Alu, X = env["Alu"], env["X"]
    ident, v_sb = env["ident"], env["v_sb"]
    ptpool, tpool, spool, psum = (env["ptpool"], env["tpool"], env["spool"],
                                  env["psum"])
    pchunks, zs, W = pk
    if h == 0:
        env["aT_tiles"][i] = tpool.tile([P, NH * 2, P], md, tag="aTt",
                                        bufs=3, name="aT_t")
    aT_t = env["aT_tiles"][i]

    nlive = i + 1
    a_ps = psum.tile([P, KC], f32, tag="ao", bufs=AO_BUFS, name="a_ps")
    gl = 0
    for (p, wc) in pchunks:
        nbl = wc // P
        trp = psum.tile([P, KC], md, tag="t", bufs=T_BUFS, name="trp")
        for j in range(nbl):
            nc.tensor.transpose(trp[:, j * P:(j + 1) * P],
                                p[:, j * P:(j + 1) * P], ident[:])
        pt = ptpool.tile([P, KC], md, tag="pt")
        nc.vector.tensor_copy(pt[:, 0:nbl * P], trp[:, 0:nbl * P])
        for j in range(nbl):
            nc.tensor.matmul(a_ps[:, 0:DH], pt[:, j * P:(j + 1) * P],
                             v_sb[:, gl, :],
                             start=(gl == 0), stop=(gl == nlive - 1))
            gl += 1

    # normalize + transpose to head-major aT via PE + one DVE copy
    z = spool.tile([P, 1], f32, tag="z", name="z")
    nc.vector.reduce_sum(z[:], zs[:, 0:W], axis=X)
    rz = spool.tile([P, 1], f32, tag="rz", name="rz")
    nc.vector.reciprocal(rz[:], z[:])
    at = tpool.tile([P, DH], md, tag="at", bufs=3)
    nc.vector.tensor_scalar_mul(at[:], a_ps[:, 0:DH], rz[:])
    atp = psum.tile([P, KC], md, tag="t", bufs=T_BUFS, name="atp")
    for e in range(2):
        nc.tensor.transpose(atp[:, e * P:(e + 1) * P],
                            at[:, e * P:(e + 1) * P], ident[:])
    nc.vector.tensor_copy(
        aT_t[:, h * 2:h * 2 + 2, :],
        atp[:, 0:2 * P].rearrange("p (b q1) -> p b q1", b=2))


def _emit_op_chain(nc, tc, env, i, dc):
    f32 = env["f32"]
    wo_sb, out_d = env["wo_sb"], env["out_d"]
    out_md = env["out_md"]
    opool, psum = env["opool"], env["psum"]
    aT_t = env["aT_tiles"][i]

    OC = 256  # half-bank chains: finer-grained PE filler
    ET = NH * DH // P  # 4
    o_ps = psum.tile([P, OC], f32, tag="ao", bufs=AO_BUFS, name="o_ps")
    for e in range(ET):
        nc.tensor.matmul(
            o_ps[:], aT_t[:, e, :],
            wo_sb[:, e, dc * OC:(dc + 1) * OC],
            start=(e == 0), stop=(e == ET - 1))
    o_sb = opool.tile([P, OC], out_md, tag="o")
    nc.scalar.copy(o_sb[:], o_ps[:])
    nc.sync.dma_start(
        out_d[i * P:(i + 1) * P, dc * OC:(dc + 1) * OC], o_sb[:])


def _can_compact(inputs):
    """Compact rope path needs all-ones norm weights, batch-identical
    cos/sin, and identical cos/sin halves (true for the reference RoPE)."""
    cos = np.asarray(inputs["cos"], np.float32)
    sin = np.asarray(inputs["sin"], np.float32)
    qnw = np.asarray(inputs["q_norm_w"], np.float32)
    knw = np.asarray(inputs["k_norm_w"], np.float32)
    hd = DH // 2
    return (np.all(qnw == 1.0) and np.all(knw == 1.0)
            and all(np.array_equal(cos[0], cos[b]) for b in range(1, B))
            and all(np.array_equal(sin[0], sin[b]) for b in range(1, B))
            and np.array_equal(cos[0][:, :hd], cos[0][:, hd:])
            and np.array_equal(sin[0][:, :hd], sin[0][:, hd:]))


def _host_prep(inputs, compact_rope=None):
    """Build the 8 per-core input maps from full inputs."""
    x = np.asarray(inputs["hidden_states"], np.float32)
    cos = np.asarray(inputs["cos"], np.float32)
    sin = np.asarray(inputs["sin"], np.float32)
    wq = np.asarray(inputs["wq"], np.float32)
    wk = np.asarray(inputs["wk"], np.float32)
    wv = np.asarray(inputs["wv"], np.float32)
    wo = np.asarray(inputs["wo"], np.float32)
    qnw = np.asarray(inputs["q_norm_w"], np.float32)
    knw = np.asarray(inputs["k_norm_w"], np.float32)

    if compact_rope is None:
        compact_rope = _can_compact(inputs)
    md = _np_md()
    hd = DH // 2

    if compact_rope:
        ch = cos[0][:, 0:hd]
        sh = sin[0][:, 0:hd]
        rope3 = [np.ascontiguousarray(
            np.concatenate([ch, -sh, sh], axis=1)).astype(md)] * B
    else:
        # rope tables with norm weight and rotate-half signs folded in
        sign = np.concatenate([-np.ones(hd), np.ones(hd)]).astype(np.float32)

        def _rope_tabs(w):
            w_rot = np.concatenate([w[hd:], w[:hd]])
            cw = [np.ascontiguousarray(cos[b] * w[None, :]).astype(md)
                  for b in range(B)]
            sw = [np.ascontiguousarray(
                      sin[b] * (sign * w_rot)[None, :]).astype(md)
                  for b in range(B)]
            return cw, sw

        cqw, sqw = _rope_tabs(qnw)
        ckw, skw = _rope_tabs(knw)

    # additive lower-triangular mask for the diagonal 128x128 block (fp32)
    r = np.arange(P)[:, None]
    c = np.arange(P)[None, :]
    trimaskf = np.where(c <= r, 0.0, NEG).astype(np.float32)

    xT = [np.ascontiguousarray(x[b].T).astype(md) for b in range(B)]

    in_maps = []
    for cid in range(8):
        b = cid // 4
        j = cid % 4
        h0 = 2 * j
        g = j // 2
        wqT = np.ascontiguousarray(wq[h0 * DH:(h0 + 2) * DH, :].T).astype(md)
        wkvT = np.ascontiguousarray(
            np.concatenate([wk[g * DH:(g + 1) * DH, :],
                            wv[g * DH:(g + 1) * DH, :]], axis=0).T).astype(md)
        woT2 = np.ascontiguousarray(wo[:, h0 * DH:(h0 + 2) * DH].T).astype(md)
        def v2(a):
            return a.view(np.uint16) if a.dtype.itemsize == 2 else a
        im = {
            "xT": v2(xT[b]),
            "wqT": v2(wqT),
            "wkvT": v2(wkvT),
            "woT2": v2(woT2),
            "trimaskf": trimaskf,
            "ident": v2(np.eye(P, dtype=md)),
        }
        if compact_rope:
            im["rope3"] = v2(rope3[b])
        else:
            im["cqw"] = v2(cqw[b])
            im["sqw"] = v2(sqw[b])
            im["ckw"] = v2(ckw[b])
            im["skw"] = v2(skw[b])
        in_maps.append(im)
    return in_maps


def kernel(**inputs) -> np.ndarray:
    compact = _can_compact(inputs)
    key = ("nc", compact)
    if key not in _cache:
        _cache[key] = _build_program(compact_rope=compact)
    nc = _cache[key]
    _cache["nc"] = nc  # last-built program, for the test harness
    in_maps = _host_prep(inputs, compact_rope=compact)
    res = bass_utils.run_bass_kernel_spmd(
        nc, in_maps, core_ids=list(range(8)))
    _cache["last_result"] = res
    out = np.zeros((B, S, D), np.float32)
    for cid in range(8):
        part = res.results[cid]["out"]
        if part.dtype == np.uint16:
            part = part.view(ml_dtypes.bfloat16).astype(np.float32)
        out[cid // 4] += part
    return out



# revision 44
# speedup vs baseline: 1.0244x; 1.0244x over previous
"""TRN2 Bass kernel for nn_AttentionModel_46823733461774.

Gemma3n-style attention block: qkv projection, q/k/v RMS-norm, RoPE on q/k,
GQA causal attention (no scaling; q_norm replaces 1/sqrt(d)), output proj.

Shapes (hardcoded): B=2, S=2048, D=2048, H=8, KV=2, DH=256.

Sharding over 8 cores: core c -> batch b=c//4, q-heads {2j, 2j+1} (j=c%4),
kv-head j//2.  Each core computes the projections for its batch/heads
(token-major), norms+RoPE, causal attention for its 2 heads, and a partial
output projection attn_heads @ wo_slice^T.  Host sums the 4 partials per
batch.  cos/sin replicated.

All matmuls in fp16 (same PE throughput as bf16, 8x the mantissa accuracy);
softmax statistics and accumulations in fp32.

Single merged pipeline: projection tiles and attention tiles are emitted
interleaved (attention for token-tile i follows projection of tile i+3), so
every engine keeps independent work during the softmax dependency chains.
Scores are computed in 512-column PSUM chunks from a 4-bank ring; the causal
mask is added to the diagonal chunk on DVE; row maxes are reduced per-chunk
on DVE as each chunk's matmuls finish; exp runs per-chunk on ACT; prob
transposes + PV accumulate chunk-by-chunk; 256-wide output-proj matmul
chains are used as fine-grained PE filler inside the softmax shadows.

Input DMA is pipelined in consumption order on the SP queue (per-queue DMA
transfers serialize): first xT chunk in dt-halves + wq quarters so the
tile-0 projection starts ~10us in, then wkv, rope, remaining xT chunks, wo.

Further structure: x^T is SBUF-resident; q^T / attn^T live in small ring
buffers; rms rsqrt is computed as exp(-0.5*ln(x)) so every ACT function
(square/ln/exp/copy) lives in one activation table (no 1.3us table
reloads); when the norm weights are all-ones (the reference setup), a
single packed half-table [cos|-sin|+sin] serves q and k rope (1.5MB instead
of 4MB of DMA + SBUF), with a full-table fallback otherwise.
"""

import os
from collections import deque

import numpy as np
import ml_dtypes

import concourse.bass as bass
import concourse.mybir as mybir
import concourse.tile as tile
from concourse import bacc
from concourse import bass_utils

B, S, D = 2, 2048, 2048
H, KV, DH = 8, 2, 256
EPS = 1e-6
NEG = -30000.0   # additive causal mask (fp16-representable; exp() -> 0)
P = 128
TT = S // P      # 16 token tiles
DT = D // P      # 16 contraction tiles
NH = 2           # heads per core
KC = 512         # key chunk (scores free dim; one PSUM bank)

# matmul dtype mode: "f16" | "bf16" | "f32"
MODE = os.environ.get("KERNEL_MODE", "f16")
# repeat the body N times inside the NEFF (for wall-clock HW timing)
ITERS = int(os.environ.get("KERNEL_ITERS", "1"))

_cache = {}


def _np_md():
    if MODE == "bf16":
        return ml_dtypes.bfloat16
    if MODE == "f16":
        return np.float16
    return np.float32


def _bir_md():
    if MODE == "bf16":
        return mybir.dt.bfloat16
    if MODE == "f16":
        return mybir.dt.float16
    return mybir.dt.float32


def _build_program(compact_rope=True):
    f32 = mybir.dt.float32
    md = _bir_md()
    Alu = mybir.AluOpType
    Act = mybir.ActivationFunctionType
    X = mybir.AxisListType.X
    XY = mybir.AxisListType.XY

    nc = bacc.Bacc("TRN2", target_bir_lowering=False, debug=False, num_devices=8)

    # fp16 buffers hang at the PJRT/axon boundary -> declare 2-byte inputs
    # as uint16 and bitcast to the matmul dtype on the DRAM APs.
    io2 = mybir.dt.uint16 if mybir.dt.size(md) == 2 else md
    def _in2(name, shape):
        ap = nc.dram_tensor(name, shape, io2, kind="ExternalInput").ap()
        return ap.bitcast(md) if io2 != md else ap
    xT_d = _in2("xT", [D, S])
    wqT_d = _in2("wqT", [D, NH * DH])
    wkvT_d = _in2("wkvT", [D, 2 * DH])
    woT2_d = _in2("woT2", [NH * DH, D])
    if compact_rope:
        # norm weights are all-ones and cos/sin halves are identical, so a
        # single packed table [S, 3*hd] = [cos_half | -sin_half | +sin_half]
        # serves q and k (1.5MB instead of 4MB of DMA + SBUF).
        rope3_d = _in2("rope3", [S, 3 * (DH // 2)])
    else:
        # rope tables with the norm weight and rotate-half signs folded in:
        # cw = cos*w ; sw[d<hd] = -sin[d]*w[d+hd], sw[d>=hd] = sin[d]*w[d-hd]
        cqw_d = _in2("cqw", [S, DH])
        sqw_d = _in2("sqw", [S, DH])
        ckw_d = _in2("ckw", [S, DH])
        skw_d = _in2("skw", [S, DH])
    trimaskf_d = nc.dram_tensor("trimaskf", [P, P], f32,
                                kind="ExternalInput").ap()
    ident_d = _in2("ident", [P, P])
    # output partials in bf16 (halves the out DMA; host sums in fp32).
    # 2-byte IO declared as uint16 like the inputs (PJRT boundary quirk).
    if io2 == md:  # f32 mode
        out_d = nc.dram_tensor("out", [S, D], f32, kind="ExternalOutput").ap()
        out_md = f32
    else:
        out_d = nc.dram_tensor("out", [S, D], mybir.dt.uint16,
                               kind="ExternalOutput").ap().bitcast(
                                   mybir.dt.bfloat16)
        out_md = mybir.dt.bfloat16

    with tile.TileContext(nc) as tc:
        with (
            tc.tile_pool(name="const", bufs=1) as cpool,
            tc.tile_pool(name="resid", bufs=1) as rpool,
            tc.tile_pool(name="pbuf", bufs=8) as ppool,
            tc.tile_pool(name="ptbuf", bufs=6) as ptpool,
            tc.tile_pool(name="tmp", bufs=10) as tpool,
            tc.tile_pool(name="stat", bufs=12) as spool,
            tc.tile_pool(name="obuf", bufs=3) as opool,
            tc.tile_pool(name="psum", bufs=1, space="PSUM") as psum,
        ):
            # ---- SBUF tiles for constants / weights / x ----
            wq_sb = cpool.tile([P, DT, NH * DH], md, tag="wq")
            wkv_sb = cpool.tile([P, DT, 2 * DH], md, tag="wkv")
            wo_sb = cpool.tile([P, NH * DH // P, D], md, tag="wo")
            xT_sb = cpool.tile([P, DT, S], md, tag="xT")
            hd = DH // 2
            if compact_rope:
                rope_sb = cpool.tile([P, TT, 3 * hd], md, tag="rope3")
            else:
                cqw_sb = cpool.tile([P, TT, DH], md, tag="cqw")
                sqw_sb = cpool.tile([P, TT, DH], md, tag="sqw")
                ckw_sb = cpool.tile([P, TT, DH], md, tag="ckw")
                skw_sb = cpool.tile([P, TT, DH], md, tag="skw")
            trif_sb = cpool.tile([P, P], f32, tag="trif")
            ident = cpool.tile([P, P], md, tag="ident")
            eps_sb = cpool.tile([P, 1], f32, tag="eps")

            # ---- pipelined input DMA, in consumption order ----
            # xT in 256-token chunks (512B contiguous runs, full DMA speed);
            # weights in halves so the first proj matmuls can start early;
            # rope tables chunked alongside the x tiles they feed; wo last
            # (first consumer is the tile-0 output chain, ~4 tiles in).
            XC = 256
            NXC = S // XC

            def load_xt(ci):
                nc.sync.dma_start(
                    xT_sb[:, :, ci * XC:(ci + 1) * XC],
                    xT_d[:, ci * XC:(ci + 1) * XC].rearrange(
                        "(dt p) t -> p dt t", p=P))

            def load_rope(ci):
                if compact_rope:
                    nc.sync.dma_start(
                        rope_sb[:, 2 * ci:2 * ci + 2, :],
                        rope3_d[ci * XC:(ci + 1) * XC, :].rearrange(
                            "(tt p) d1 -> p tt d1", p=P))
                else:
                    for sb, dr in ((cqw_sb, cqw_d), (sqw_sb, sqw_d),
                                   (ckw_sb, ckw_d), (skw_sb, skw_d)):
                        nc.sync.dma_start(
                            sb[:, 2 * ci:2 * ci + 2, :],
                            dr[ci * XC:(ci + 1) * XC, :].rearrange(
                                "(tt p) d1 -> p tt d1", p=P))

            # first xT chunk in dt-halves interleaved with wq quarters, so
            # the tile-0 q-projection chain starts as soon as the first
            # 0.5MB pieces land instead of after 2.5MB
            QDT = DT // 4
            def load_wq_q(qi):
                nc.sync.dma_start(
                    wq_sb[:, qi * QDT:(qi + 1) * QDT, :],
                    wqT_d[qi * D // 4:(qi + 1) * D // 4, :].rearrange(
                        "(dt p) e -> p dt e", p=P))

            nc.sync.dma_start(
                xT_sb[:, 0:DT // 2, 0:XC],
                xT_d[0:D // 2, 0:XC].rearrange("(dt p) t -> p dt t", p=P))
            load_wq_q(0)
            load_wq_q(1)
            nc.sync.dma_start(
                xT_sb[:, DT // 2:DT, 0:XC],
                xT_d[D // 2:D, 0:XC].rearrange("(dt p) t -> p dt t", p=P))
            load_wq_q(2)
            load_wq_q(3)
            HDT = DT // 2
            for h2 in range(2):
                nc.sync.dma_start(
                    wkv_sb[:, h2 * HDT:(h2 + 1) * HDT, :],
                    wkvT_d[h2 * D // 2:(h2 + 1) * D // 2, :].rearrange(
                        "(dt p) e -> p dt e", p=P))
            nc.sync.dma_start(trif_sb[:], trimaskf_d)
            nc.sync.dma_start(ident[:], ident_d)
            nc.gpsimd.memset(eps_sb[:], EPS)
            load_rope(0)
            for ci in range(1, 4):
                load_xt(ci)
                load_rope(ci)
            nc.sync.dma_start(
                wo_sb[:], woT2_d.rearrange("(et p) d1 -> p et d1", p=P))
            for ci in range(4, NXC):
                load_xt(ci)
                load_rope(ci)

            # ---- persistent activations ----
            kT_sb = rpool.tile([P, 2, S], md, tag="kT")
            v_sb = rpool.tile([P, TT, DH], md, tag="v")        # token-major

            env = dict(
                f32=f32, md=md, Alu=Alu, Act=Act, X=X, XY=XY,
                wq_sb=wq_sb, wkv_sb=wkv_sb, wo_sb=wo_sb, trif_sb=trif_sb,
                ident=ident, eps_sb=eps_sb, kT_sb=kT_sb,
                v_sb=v_sb, xT_sb=xT_sb, out_d=out_d, out_md=out_md,
                ppool=ppool, ptpool=ptpool, tpool=tpool,
                spool=spool, opool=opool, psum=psum,
                qT_tiles={}, aT_tiles={},
            )
            if compact_rope:
                env["rope_sb"] = rope_sb
            else:
                env.update(cqw_sb=cqw_sb, sqw_sb=sqw_sb,
                           ckw_sb=ckw_sb, skw_sb=skw_sb)
            env["compact_rope"] = compact_rope
            import contextlib
            unroll = int(os.environ.get("KERNEL_UNROLL", "1"))
            loop_ctx = (tc.For_i(0, ITERS, 1) if ITERS > 1
                        else contextlib.nullcontext())
            with loop_ctx:
                for _ in range(unroll):
                    _emit_body(nc, tc, env)

    # Activation-table pre-placement: the stock pass greedily maps each
    # activation to the FIRST act_info.json set containing its function,
    # which ping-pongs between the exp table and the ln table (1.3us reload
    # each).  Pre-place loads with our functions masked out of every set
    # before natural_log_exp_and_others, so everything first-matches that
    # one set (it contains exp+ln+square+copy+identity) and a single load
    # suffices.  Set ids keep their original act_info.json indices.
    from concourse.hw_specs import get_activation_tables
    import bass_rust as _br
    Act = mybir.ActivationFunctionType
    tables = list(get_activation_tables(nc.m.arch).items())
    target = next(idx for idx, (n, s) in enumerate(tables)
                  if n == "natural_log_exp_and_others")
    ours = {Act.Square, Act.Ln, Act.Exp, Act.Copy, Act.Identity}
    tables = [(n, (s - ours) if idx < target else s)
              for idx, (n, s) in enumerate(tables)]
    _br.insert_act_table_loads(nc, tables)

    nc.compile()
    return nc


# PSUM bank budget (8 banks): score-chunk ring "s" x4, transpose landing
# "t" x2, PV-accumulator / out-proj chain ring "ao" x2.
S_BUFS = 4
T_BUFS = 2
AO_BUFS = 2
PRE = 3  # attention for tile i is emitted after projection of tile i+PRE


def _emit_body(nc, tc, env):
    fillers = deque()

    def pop_fill(n):
        for _ in range(min(n, len(fillers))):
            fillers.popleft()()

    for t in range(TT + PRE):
        i = t - PRE
        pj = _emit_proj_q(nc, tc, env, t) if t < TT else None
        if pj is not None:
            _emit_proj_kv(nc, tc, env, t, pj)
        if 0 <= i < TT:
            # i-scaled filler: bank surplus chains early, spend them on the
            # longer softmax shadows of late (wide-W) tiles
            npop = 3 if i < 8 else (4 if i < 13 else 6)
            pk = _emit_attn_scores(nc, tc, env, i, 0)
            pop_fill(npop)
            _emit_attn_rest(nc, tc, env, i, 0, pk)
            if pj is not None:
                _emit_proj_rope(nc, tc, env, t, pj, part=0)
            pk = _emit_attn_scores(nc, tc, env, i, 1)
            pop_fill(npop)
            _emit_attn_rest(nc, tc, env, i, 1, pk)
            if pj is not None:
                _emit_proj_rope(nc, tc, env, t, pj, part=1)
            for dc in range(D // 256):
                fillers.append(
                    lambda i=i, dc=dc: _emit_op_chain(nc, tc, env, i, dc))
        elif pj is not None:
            _emit_proj_rope(nc, tc, env, t, pj, part=0)
            _emit_proj_rope(nc, tc, env, t, pj, part=1)
    pop_fill(len(fillers))


def _stat_pair(nc, env, srcs, pair):
    """rr = exp(-0.5*ln(mean(x^2)+eps)) for a pair of DH-wide sources.
    Ln+Exp share an ACT function table with Square/Copy: no table reloads."""
    f32, Act = env["f32"], env["Act"]
    tpool, spool = env["tpool"], env["spool"]
    ss2 = spool.tile([P, 2], f32, tag=f"ss{pair}", name="ss2")
    for j in (0, 1):
        sq = tpool.tile([P, DH], f32, tag="sq", bufs=2)
        nc.scalar.activation(sq[:], srcs[j], Act.Square,
                             accum_out=ss2[:, j:j + 1])
    lg2 = spool.tile([P, 2], f32, tag=f"lg{pair}", name="lg2")
    nc.scalar.activation(lg2[:], ss2[:], Act.Ln,
                         bias=env["eps_sb"][:], scale=1.0 / DH)
    rr2 = spool.tile([P, 2], f32, tag=f"rr{pair}", name="rr2")
    nc.scalar.activation(rr2[:], lg2[:], Act.Exp, scale=-0.5)
    return rr2


def _emit_proj_q(nc, tc, env, tt):
    """q projection matmuls for token tile tt + PSUM->SBUF evacuation + rms
    statistics.  The kv half is emitted separately (inside the h0 softmax
    shadow) via _emit_proj_kv."""
    f32, md = env["f32"], env["md"]
    xT_sb = env["xT_sb"]
    tpool, psum = env["tpool"], env["psum"]

    q_ps = psum.tile([P, NH * DH], f32, tag="s", bufs=S_BUFS, name="q_ps")
    for d in range(DT):
        nc.tensor.matmul(q_ps[:], xT_sb[:, d, tt * P:(tt + 1) * P],
                         env["wq_sb"][:, d, :],
                         start=(d == 0), stop=(d == DT - 1))
    qsb = tpool.tile([P, NH * DH], md, tag="qsb", bufs=3)
    nc.scalar.copy(qsb[:], q_ps[:])
    rr_q = _stat_pair(nc, env, [qsb[:, 0:DH], qsb[:, DH:2 * DH]], 0)
    return dict(qsb=qsb, rr_q=rr_q)


def _emit_proj_kv(nc, tc, env, tt, pj):
    f32, md = env["f32"], env["md"]
    xT_sb = env["xT_sb"]
    tpool, psum = env["tpool"], env["psum"]

    kv_ps = psum.tile([P, 2 * DH], f32, tag="s", bufs=S_BUFS, name="kv_ps")
    for d in range(DT):
        nc.tensor.matmul(kv_ps[:], xT_sb[:, d, tt * P:(tt + 1) * P],
                         env["wkv_sb"][:, d, :],
                         start=(d == 0), stop=(d == DT - 1))
    kvsb = tpool.tile([P, 2 * DH], md, tag="kvsb", bufs=3)
    nc.scalar.copy(kvsb[:], kv_ps[:])
    rr_kv = _stat_pair(nc, env, [kvsb[:, 0:DH], kvsb[:, DH:2 * DH]], 1)
    pj["kvsb"] = kvsb
    pj["rr_kv"] = rr_kv


def _emit_proj_rope(nc, tc, env, tt, pj, part):
    """part 0: q heads norm+rope; part 1: k norm+rope and v norm.  rope+norm
    in ~5 DVE ops per head:
      u  = (src * rr) * c           (c = cos table)
      v  = (rot_half(src) * rr) * s   (signs folded into s; 2 half-ops)
      qr = u + v
    Head-major transposes run on the DMA engines (XBAR 128-block transpose),
    keeping the PE free."""
    f32, md = env["f32"], env["md"]
    Alu = env["Alu"]
    kT_sb, v_sb = env["kT_sb"], env["v_sb"]
    tpool = env["tpool"]

    hd = DH // 2
    compact = env["compact_rope"]
    whichs = (0, 1) if part == 0 else (2,)
    psum = env["psum"]
    ident = env["ident"]
    qr2 = tpool.tile([P, len(whichs) * DH], md,
                     tag="qr2" if part == 0 else "kr2", bufs=3)
    for slot, which in enumerate(whichs):
        if which < NH:
            src = pj["qsb"][:, which * DH:(which + 1) * DH]
            rr = pj["rr_q"][:, which:which + 1]
        else:
            src = pj["kvsb"][:, 0:DH]
            rr = pj["rr_kv"][:, 0:1]
        if compact:
            ch = env["rope_sb"][:, tt, 0:hd]
            sn = env["rope_sb"][:, tt, hd:2 * hd]
            sp = env["rope_sb"][:, tt, 2 * hd:3 * hd]
        else:
            if which < NH:
                cw = env["cqw_sb"][:, tt, :]
                sw = env["sqw_sb"][:, tt, :]
            else:
                cw = env["ckw_sb"][:, tt, :]
                sw = env["skw_sb"][:, tt, :]
        u = tpool.tile([P, DH], md, tag="qa", bufs=3)
        if compact:
            nc.vector.scalar_tensor_tensor(u[:, 0:hd], src[:, 0:hd], rr, ch,
                                           op0=Alu.mult, op1=Alu.mult)
            nc.vector.scalar_tensor_tensor(u[:, hd:DH], src[:, hd:DH], rr, ch,
                                           op0=Alu.mult, op1=Alu.mult)
        else:
            nc.vector.scalar_tensor_tensor(u[:], src, rr, cw,
                                           op0=Alu.mult, op1=Alu.mult)
        v = tpool.tile([P, DH], md, tag="t1", bufs=3)
        nc.vector.scalar_tensor_tensor(v[:, 0:hd], src[:, hd:DH], rr,
                                       sn if compact else sw[:, 0:hd],
                                       op0=Alu.mult, op1=Alu.mult)
        nc.vector.scalar_tensor_tensor(v[:, hd:DH], src[:, 0:hd], rr,
                                       sp if compact else sw[:, hd:DH],
                                       op0=Alu.mult, op1=Alu.mult)
        nc.vector.tensor_add(qr2[:, slot * DH:(slot + 1) * DH], u[:], v[:])
    if True:
        # PE transpose into head-major layout, then one batched DVE copy
        nblk = 2 * len(whichs)
        tp_ps = psum.tile([P, 512], md, tag="t", bufs=T_BUFS, name="tp_ps")
        for blk in range(nblk):
            nc.tensor.transpose(tp_ps[:, blk * P:(blk + 1) * P],
                                qr2[:, blk * P:(blk + 1) * P], ident[:])
        if part == 0:
            qT_t = tpool.tile([P, NH * 2, P], md, tag="qTt", bufs=5,
                              name="qT_t")
            nc.vector.tensor_copy(
                qT_t[:], tp_ps[:].rearrange("p (b q1) -> p b q1", b=4))
            env["qT_tiles"][tt] = qT_t
        else:
            nc.vector.tensor_copy(
                kT_sb[:, :, tt * P:(tt + 1) * P],
                tp_ps[:, 0:2 * P].rearrange("p (b q1) -> p b q1", b=2))
    if part == 1:
        # ---- v: rms-norm only, stays token-major (fp16 SBUF -> 4x mode)
        nc.vector.tensor_scalar_mul(v_sb[:, tt, :], pj["kvsb"][:, DH:2 * DH],
                                    pj["rr_kv"][:, 1:2])


def _emit_attn_scores(nc, tc, env, i, h):
    """Scores in 512-col PSUM chunks + per-chunk max + exp.  Returns the
    packet (pchunks, zs, W) consumed by _emit_attn_rest."""
    f32, md = env["f32"], env["md"]
    Alu, Act, X = env["Alu"], env["Act"], env["X"]
    kT_sb = env["kT_sb"]
    trif_sb = env["trif_sb"]
    ppool, spool, psum = env["ppool"], env["spool"], env["psum"]
    qT_t = env["qT_tiles"][i]

    nlive = i + 1
    W = (nlive * P + KC - 1) // KC
    mxs = spool.tile([P, 5], f32, tag="mxs", name="mxs")
    schunks = []
    for c in range(W):
        k0 = c * KC
        k1 = min((c + 1) * KC, nlive * P)
        wc = k1 - k0
        s = psum.tile([P, KC], f32, tag="s", bufs=S_BUFS, name="s")
        for dh in (0, 1):
            nc.tensor.matmul(
                s[:, 0:wc], qT_t[:, h * 2 + dh, :],
                kT_sb[:, dh, k0:k1],
                start=(dh == 0), stop=(dh == 1))
        if c == W - 1:
            # additive causal mask on the diagonal 128x128 block (DVE)
            nc.vector.tensor_tensor(s[:, wc - P:wc], s[:, wc - P:wc],
                                    trif_sb[:], op=Alu.add)
        nc.vector.tensor_reduce(mxs[:, c:c + 1], s[:, 0:wc], axis=X,
                                op=Alu.max, negate=True)
        schunks.append((s, wc))
    negm = spool.tile([P, 1], f32, tag="negm", name="negm")
    nc.vector.tensor_reduce(negm[:], mxs[:, 0:W], axis=X, op=Alu.min)

    zs = spool.tile([P, 4], f32, tag="zs", name="zs")
    pchunks = []
    for c, (s, wc) in enumerate(schunks):
        p = ppool.tile([P, KC], md, tag="p")
        nc.scalar.activation(p[:, 0:wc], s[:, 0:wc], Act.Exp,
                             bias=negm[:], accum_out=zs[:, c:c + 1])
        pchunks.append((p, wc))
    return (pchunks, zs, W)


def _emit_attn_rest(nc, tc, env, i, h, pk):
    f32, md = env["f32"], env["md"]
    Alu, X = env["Alu"], env["X"]
    ident, v_sb = env["ident"], env["v_sb"]
    ptpool, tpool, spool, psum = (env["ptpool"], env["tpool"], env["spool"],
                                  env["psum"])
    pchunks, zs, W = pk
    if h == 0:
        env["aT_tiles"][i] = tpool.tile([P, NH * 2, P], md, tag="aTt",
                                        bufs=3, name="aT_t")
    aT_t = env["aT_tiles"][i]

    nlive = i + 1
    a_ps = psum.tile([P, KC], f32, tag="ao", bufs=AO_BUFS, name="a_ps")
    gl = 0
    for (p, wc) in pchunks:
        nbl = wc // P
        trp = psum.tile([P, KC], md, tag="t", bufs=T_BUFS, name="trp")
        for j in range(nbl):
            nc.tensor.transpose(trp[:, j * P:(j + 1) * P],
                                p[:, j * P:(j + 1) * P], ident[:])
        pt = ptpool.tile([P, KC], md, tag="pt")
        nc.vector.tensor_copy(pt[:, 0:nbl * P], trp[:, 0:nbl * P])
        for j in range(nbl):
            nc.tensor.matmul(a_ps[:, 0:DH], pt[:, j * P:(j + 1) * P],
                             v_sb[:, gl, :],
                             start=(gl == 0), stop=(gl == nlive - 1))
            gl += 1

    # normalize + transpose to head-major aT
    z = spool.tile([P, 1], f32, tag="z", name="z")
    nc.vector.reduce_sum(z[:], zs[:, 0:W], axis=X)
    rz = spool.tile([P, 1], f32, tag="rz", name="rz")
    nc.vector.reciprocal(rz[:], z[:])
    if True:
        at = tpool.tile([P, DH], md, tag="at", bufs=3)
        nc.vector.tensor_scalar_mul(at[:], a_ps[:, 0:DH], rz[:])
        atp = psum.tile([P, KC], md, tag="t", bufs=T_BUFS, name="atp")
        for e in range(2):
            nc.tensor.transpose(atp[:, e * P:(e + 1) * P],
                                at[:, e * P:(e + 1) * P], ident[:])
        nc.vector.tensor_copy(
            aT_t[:, h * 2:h * 2 + 2, :],
            atp[:, 0:2 * P].rearrange("p (b q1) -> p b q1", b=2))


def _emit_op_chain(nc, tc, env, i, dc):
    f32 = env["f32"]
    wo_sb, out_d = env["wo_sb"], env["out_d"]
    out_md = env["out_md"]
    opool, psum = env["opool"], env["psum"]
    aT_t = env["aT_tiles"][i]

    OC = 256  # half-bank chains: finer-grained PE filler
    ET = NH * DH // P  # 4
    o_ps = psum.tile([P, OC], f32, tag="ao", bufs=AO_BUFS, name="o_ps")
    for e in range(ET):
        nc.tensor.matmul(
            o_ps[:], aT_t[:, e, :],
            wo_sb[:, e, dc * OC:(dc + 1) * OC],
            start=(e == 0), stop=(e == ET - 1))
    o_sb = opool.tile([P, OC], out_md, tag="o")
    nc.scalar.copy(o_sb[:], o_ps[:])
    nc.sync.dma_start(
        out_d[i * P:(i + 1) * P, dc * OC:(dc + 1) * OC], o_sb[:])


def _can_compact(inputs):
    """Compact rope path needs all-ones norm weights, batch-identical
    cos/sin, and identical cos/sin halves (true for the reference RoPE)."""
    cos = np.asarray(inputs["cos"], np.float32)
    sin = np.asarray(inputs["sin"], np.float32)
    qnw = np.asarray(inputs["q_norm_w"], np.float32)
    knw = np.asarray(inputs["k_norm_w"], np.float32)
    hd = DH // 2
    return (np.all(qnw == 1.0) and np.all(knw == 1.0)
            and all(np.array_equal(cos[0], cos[b]) for b in range(1, B))
            and all(np.array_equal(sin[0], sin[b]) for b in range(1, B))
            and np.array_equal(cos[0][:, :hd], cos[0][:, hd:])
            and np.array_equal(sin[0][:, :hd], sin[0][:, hd:]))


def _host_prep(inputs, compact_rope=None):
    """Build the 8 per-core input maps from full inputs."""
    x = np.asarray(inputs["hidden_states"], np.float32)
    cos = np.asarray(inputs["cos"], np.float32)
    sin = np.asarray(inputs["sin"], np.float32)
    wq = np.asarray(inputs["wq"], np.float32)
    wk = np.asarray(inputs["wk"], np.float32)
    wv = np.asarray(inputs["wv"], np.float32)
    wo = np.asarray(inputs["wo"], np.float32)
    qnw = np.asarray(inputs["q_norm_w"], np.float32)
    knw = np.asarray(inputs["k_norm_w"], np.float32)

    if compact_rope is None:
        compact_rope = _can_compact(inputs)
    md = _np_md()
    hd = DH // 2

    if compact_rope:
        ch = cos[0][:, 0:hd]
        sh = sin[0][:, 0:hd]
        rope3 = [np.ascontiguousarray(
            np.concatenate([ch, -sh, sh], axis=1)).astype(md)] * B
    else:
        # rope tables with norm weight and rotate-half signs folded in
        sign = np.concatenate([-np.ones(hd), np.ones(hd)]).astype(np.float32)

        def _rope_tabs(w):
            w_rot = np.concatenate([w[hd:], w[:hd]])
            cw = [np.ascontiguousarray(cos[b] * w[None, :]).astype(md)
                  for b in range(B)]
            sw = [np.ascontiguousarray(
                      sin[b] * (sign * w_rot)[None, :]).astype(md)
                  for b in range(B)]
            return cw, sw

        cqw, sqw = _rope_tabs(qnw)
        ckw, skw = _rope_tabs(knw)

    # additive lower-triangular mask for the diagonal 128x128 block (fp32)
    r = np.arange(P)[:, None]
    c = np.arange(P)[None, :]
    trimaskf = np.where(c <= r, 0.0, NEG).astype(np.float32)

    xT = [np.ascontiguousarray(x[b].T).astype(md) for b in range(B)]

    in_maps = []
    for cid in range(8):
        b = cid // 4
        j = cid % 4
        h0 = 2 * j
        g = j // 2
        wqT = np.ascontiguousarray(wq[h0 * DH:(h0 + 2) * DH, :].T).astype(md)
        wkvT = np.ascontiguousarray(
            np.concatenate([wk[g * DH:(g + 1) * DH, :],
                            wv[g * DH:(g + 1) * DH, :]], axis=0).T).astype(md)
        woT2 = np.ascontiguousarray(wo[:, h0 * DH:(h0 + 2) * DH].T).astype(md)
        def v2(a):
            return a.view(np.uint16) if a.dtype.itemsize == 2 else a
        im = {
            "xT": v2(xT[b]),
            "wqT": v2(wqT),
            "wkvT": v2(wkvT),
            "woT2": v2(woT2),
            "trimaskf": trimaskf,
            "ident": v2(np.eye(P, dtype=md)),
        }
        if compact_rope:
            im["rope3"] = v2(rope3[b])
        else:
            im["cqw"] = v2(cqw[b])
            im["sqw"] = v2(sqw[b])
            im["ckw"] = v2(ckw[b])
            im["skw"] = v2(skw[b])
        in_maps.append(im)
    return in_maps


def kernel(**inputs) -> np.ndarray:
    compact = _can_compact(inputs)
    key = ("nc", compact)
    if key not in _cache:
        _cache[key] = _build_program(compact_rope=compact)
    nc = _cache[key]
    _cache["nc"] = nc  # last-built program, for the test harness
    in_maps = _host_prep(inputs, compact_rope=compact)
    res = bass_utils.run_bass_kernel_spmd(
        nc, in_maps, core_ids=list(range(8)))
    _cache["last_result"] = res
    out = np.zeros((B, S, D), np.float32)
    for cid in range(8):
        part = res.results[cid]["out"]
        if part.dtype == np.uint16:
            part = part.view(ml_dtypes.bfloat16).astype(np.float32)
        out[cid // 4] += part
    return out



# revision 46
# speedup vs baseline: 1.0264x; 1.0020x over previous
"""TRN2 Bass kernel for nn_AttentionModel_46823733461774.

Gemma3n-style attention block: qkv projection, q/k/v RMS-norm, RoPE on q/k,
GQA causal attention (no scaling; q_norm replaces 1/sqrt(d)), output proj.

Shapes (hardcoded): B=2, S=2048, D=2048, H=8, KV=2, DH=256.

Sharding over 8 cores: core c -> batch b=c//4, q-heads {2j, 2j+1} (j=c%4),
kv-head j//2.  Each core computes the projections for its batch/heads
(token-major), norms+RoPE, causal attention for its 2 heads, and a partial
output projection attn_heads @ wo_slice^T.  Host sums the 4 partials per
batch.  cos/sin replicated.

All matmuls in fp16 (same PE throughput as bf16, 8x the mantissa accuracy);
softmax statistics and accumulations in fp32.

Single merged pipeline: projection tiles and attention tiles are emitted
interleaved (attention for token-tile i follows projection of tile i+3), so
every engine keeps independent work during the softmax dependency chains.
Scores are computed in 512-column PSUM chunks from a 4-bank ring; the causal
mask is added to the diagonal chunk on DVE; row maxes are reduced per-chunk
on DVE as each chunk's matmuls finish; exp runs per-chunk on ACT; prob
transposes + PV accumulate chunk-by-chunk; 256-wide output-proj matmul
chains are used as fine-grained PE filler inside the softmax shadows.

Input DMA is pipelined in consumption order on the SP queue (per-queue DMA
transfers serialize): first xT chunk in dt-halves + wq quarters so the
tile-0 projection starts ~10us in, then wkv, rope, remaining xT chunks, wo.

Further structure: x^T is SBUF-resident; q^T / attn^T live in small ring
buffers; rms rsqrt is computed as exp(-0.5*ln(x)) so every ACT function
(square/ln/exp/copy) lives in one activation table (no 1.3us table
reloads); when the norm weights are all-ones (the reference setup), a
single packed half-table [cos|-sin|+sin] serves q and k rope (1.5MB instead
of 4MB of DMA + SBUF), with a full-table fallback otherwise.
"""

import os
from collections import deque

import numpy as np
import ml_dtypes

import concourse.bass as bass
import concourse.mybir as mybir
import concourse.tile as tile
from concourse import bacc
from concourse import bass_utils

B, S, D = 2, 2048, 2048
H, KV, DH = 8, 2, 256
EPS = 1e-6
NEG = -30000.0   # additive causal mask (fp16-representable; exp() -> 0)
P = 128
TT = S // P      # 16 token tiles
DT = D // P      # 16 contraction tiles
NH = 2           # heads per core
KC = 512         # key chunk (scores free dim; one PSUM bank)

# matmul dtype mode: "f16" | "bf16" | "f32"
MODE = os.environ.get("KERNEL_MODE", "f16")
# repeat the body N times inside the NEFF (for wall-clock HW timing)
ITERS = int(os.environ.get("KERNEL_ITERS", "1"))

_cache = {}


def _np_md():
    if MODE == "bf16":
        return ml_dtypes.bfloat16
    if MODE == "f16":
        return np.float16
    return np.float32


def _bir_md():
    if MODE == "bf16":
        return mybir.dt.bfloat16
    if MODE == "f16":
        return mybir.dt.float16
    return mybir.dt.float32


def _build_program(compact_rope=True):
    f32 = mybir.dt.float32
    md = _bir_md()
    Alu = mybir.AluOpType
    Act = mybir.ActivationFunctionType
    X = mybir.AxisListType.X
    XY = mybir.AxisListType.XY

    nc = bacc.Bacc("TRN2", target_bir_lowering=False, debug=False, num_devices=8)

    # fp16 buffers hang at the PJRT/axon boundary -> declare 2-byte inputs
    # as uint16 and bitcast to the matmul dtype on the DRAM APs.
    io2 = mybir.dt.uint16 if mybir.dt.size(md) == 2 else md
    def _in2(name, shape):
        ap = nc.dram_tensor(name, shape, io2, kind="ExternalInput").ap()
        return ap.bitcast(md) if io2 != md else ap
    xT_d = _in2("xT", [D, S])
    wqT_d = _in2("wqT", [D, NH * DH])
    wkvT_d = _in2("wkvT", [D, 2 * DH])
    woT2_d = _in2("woT2", [NH * DH, D])
    if compact_rope:
        # norm weights are all-ones and cos/sin halves are identical, so a
        # single packed table [S, 3*hd] = [cos_half | -sin_half | +sin_half]
        # serves q and k (1.5MB instead of 4MB of DMA + SBUF).
        rope3_d = _in2("rope3", [S, 3 * (DH // 2)])
    else:
        # rope tables with the norm weight and rotate-half signs folded in:
        # cw = cos*w ; sw[d<hd] = -sin[d]*w[d+hd], sw[d>=hd] = sin[d]*w[d-hd]
        cqw_d = _in2("cqw", [S, DH])
        sqw_d = _in2("sqw", [S, DH])
        ckw_d = _in2("ckw", [S, DH])
        skw_d = _in2("skw", [S, DH])
    trimaskf_d = nc.dram_tensor("trimaskf", [P, P], f32,
                                kind="ExternalInput").ap()
    ident_d = _in2("ident", [P, P])
    # output partials in bf16 (halves the out DMA; host sums in fp32).
    # 2-byte IO declared as uint16 like the inputs (PJRT boundary quirk).
    if io2 == md:  # f32 mode
        out_d = nc.dram_tensor("out", [S, D], f32, kind="ExternalOutput").ap()
        out_md = f32
    else:
        out_d = nc.dram_tensor("out", [S, D], mybir.dt.uint16,
                               kind="ExternalOutput").ap().bitcast(
                                   mybir.dt.bfloat16)
        out_md = mybir.dt.bfloat16

    with tile.TileContext(nc) as tc:
        with (
            tc.tile_pool(name="const", bufs=1) as cpool,
            tc.tile_pool(name="resid", bufs=1) as rpool,
            tc.tile_pool(name="pbuf", bufs=8) as ppool,
            tc.tile_pool(name="ptbuf", bufs=6) as ptpool,
            tc.tile_pool(name="tmp", bufs=10) as tpool,
            tc.tile_pool(name="stat", bufs=12) as spool,
            tc.tile_pool(name="obuf", bufs=3) as opool,
            tc.tile_pool(name="psum", bufs=1, space="PSUM") as psum,
        ):
            # ---- SBUF tiles for constants / weights / x ----
            wq_sb = cpool.tile([P, DT, NH * DH], md, tag="wq")
            wkv_sb = cpool.tile([P, DT, 2 * DH], md, tag="wkv")
            wo_sb = cpool.tile([P, NH * DH // P, D], md, tag="wo")
            xT_sb = cpool.tile([P, DT, S], md, tag="xT")
            hd = DH // 2
            if compact_rope:
                rope_sb = cpool.tile([P, TT, 3 * hd], md, tag="rope3")
            else:
                cqw_sb = cpool.tile([P, TT, DH], md, tag="cqw")
                sqw_sb = cpool.tile([P, TT, DH], md, tag="sqw")
                ckw_sb = cpool.tile([P, TT, DH], md, tag="ckw")
                skw_sb = cpool.tile([P, TT, DH], md, tag="skw")
            trif_sb = cpool.tile([P, P], f32, tag="trif")
            ident = cpool.tile([P, P], md, tag="ident")
            eps_sb = cpool.tile([P, 1], f32, tag="eps")

            # ---- pipelined input DMA, in consumption order ----
            # xT in 256-token chunks (512B contiguous runs, full DMA speed);
            # weights in halves so the first proj matmuls can start early;
            # rope tables chunked alongside the x tiles they feed; wo last
            # (first consumer is the tile-0 output chain, ~4 tiles in).
            XC = 256
            NXC = S // XC

            def load_xt(ci):
                nc.sync.dma_start(
                    xT_sb[:, :, ci * XC:(ci + 1) * XC],
                    xT_d[:, ci * XC:(ci + 1) * XC].rearrange(
                        "(dt p) t -> p dt t", p=P))

            def load_rope(ci):
                if compact_rope:
                    nc.sync.dma_start(
                        rope_sb[:, 2 * ci:2 * ci + 2, :],
                        rope3_d[ci * XC:(ci + 1) * XC, :].rearrange(
                            "(tt p) d1 -> p tt d1", p=P))
                else:
                    for sb, dr in ((cqw_sb, cqw_d), (sqw_sb, sqw_d),
                                   (ckw_sb, ckw_d), (skw_sb, skw_d)):
                        nc.sync.dma_start(
                            sb[:, 2 * ci:2 * ci + 2, :],
                            dr[ci * XC:(ci + 1) * XC, :].rearrange(
                                "(tt p) d1 -> p tt d1", p=P))

            # first xT chunk in dt-halves interleaved with wq quarters, so
            # the tile-0 q-projection chain starts as soon as the first
            # 0.5MB pieces land instead of after 2.5MB
            QDT = DT // 4
            def load_wq_q(qi):
                nc.sync.dma_start(
                    wq_sb[:, qi * QDT:(qi + 1) * QDT, :],
                    wqT_d[qi * D // 4:(qi + 1) * D // 4, :].rearrange(
                        "(dt p) e -> p dt e", p=P))

            nc.sync.dma_start(
                xT_sb[:, 0:DT // 2, 0:XC],
                xT_d[0:D // 2, 0:XC].rearrange("(dt p) t -> p dt t", p=P))
            load_wq_q(0)
            load_wq_q(1)
            nc.sync.dma_start(
                xT_sb[:, DT // 2:DT, 0:XC],
                xT_d[D // 2:D, 0:XC].rearrange("(dt p) t -> p dt t", p=P))
            load_wq_q(2)
            load_wq_q(3)
            HDT = DT // 2
            for h2 in range(2):
                nc.sync.dma_start(
                    wkv_sb[:, h2 * HDT:(h2 + 1) * HDT, :],
                    wkvT_d[h2 * D // 2:(h2 + 1) * D // 2, :].rearrange(
                        "(dt p) e -> p dt e", p=P))
            nc.sync.dma_start(trif_sb[:], trimaskf_d)
            nc.sync.dma_start(ident[:], ident_d)
            nc.gpsimd.memset(eps_sb[:], EPS)
            load_rope(0)
            for ci in range(1, 4):
                load_xt(ci)
                load_rope(ci)
            nc.sync.dma_start(
                wo_sb[:], woT2_d.rearrange("(et p) d1 -> p et d1", p=P))
            for ci in range(4, NXC):
                load_xt(ci)
                load_rope(ci)

            # ---- persistent activations ----
            kT_sb = rpool.tile([P, 2, S], md, tag="kT")
            v_sb = rpool.tile([P, TT, DH], md, tag="v")        # token-major

            env = dict(
                f32=f32, md=md, Alu=Alu, Act=Act, X=X, XY=XY,
                wq_sb=wq_sb, wkv_sb=wkv_sb, wo_sb=wo_sb, trif_sb=trif_sb,
                ident=ident, eps_sb=eps_sb, kT_sb=kT_sb,
                v_sb=v_sb, xT_sb=xT_sb, out_d=out_d, out_md=out_md,
                ppool=ppool, ptpool=ptpool, tpool=tpool,
                spool=spool, opool=opool, psum=psum,
                qT_tiles={}, aT_tiles={},
            )
            if compact_rope:
                env["rope_sb"] = rope_sb
            else:
                env.update(cqw_sb=cqw_sb, sqw_sb=sqw_sb,
                           ckw_sb=ckw_sb, skw_sb=skw_sb)
            env["compact_rope"] = compact_rope
            import contextlib
            unroll = int(os.environ.get("KERNEL_UNROLL", "1"))
            loop_ctx = (tc.For_i(0, ITERS, 1) if ITERS > 1
                        else contextlib.nullcontext())
            with loop_ctx:
                for _ in range(unroll):
                    _emit_body(nc, tc, env)

    # Activation-table pre-placement: the stock pass greedily maps each
    # activation to the FIRST act_info.json set containing its function,
    # which ping-pongs between the exp table and the ln table (1.3us reload
    # each).  Pre-place loads with our functions masked out of every set
    # before natural_log_exp_and_others, so everything first-matches that
    # one set (it contains exp+ln+square+copy+identity) and a single load
    # suffices.  Set ids keep their original act_info.json indices.
    from concourse.hw_specs import get_activation_tables
    import bass_rust as _br
    Act = mybir.ActivationFunctionType
    tables = list(get_activation_tables(nc.m.arch).items())
    target = next(idx for idx, (n, s) in enumerate(tables)
                  if n == "natural_log_exp_and_others")
    ours = {Act.Square, Act.Ln, Act.Exp, Act.Copy, Act.Identity}
    tables = [(n, (s - ours) if idx < target else s)
              for idx, (n, s) in enumerate(tables)]
    _br.insert_act_table_loads(nc, tables)

    nc.compile()
    return nc


# PSUM bank budget (8 banks): score-chunk ring "s" x4, transpose landing
# "t" x2, PV-accumulator / out-proj chain ring "ao" x2.
S_BUFS = 4
T_BUFS = 2
AO_BUFS = 2
PRE = 3  # attention for tile i is emitted after projection of tile i+PRE


def _emit_body(nc, tc, env):
    fillers = deque()

    def pop_fill(n):
        for _ in range(min(n, len(fillers))):
            fillers.popleft()()

    for t in range(TT + PRE):
        i = t - PRE
        pj = _emit_proj_q(nc, tc, env, t) if t < TT else None
        if pj is not None:
            _emit_proj_kv(nc, tc, env, t, pj)
        if 0 <= i < TT:
            # i-scaled filler: bank surplus chains early, spend them on the
            # longer softmax shadows of late (wide-W) tiles
            npop = 3 if i < 8 else (4 if i < 13 else 6)
            pk = _emit_attn_scores(nc, tc, env, i, 0)
            pop_fill(npop)
            _emit_attn_rest(nc, tc, env, i, 0, pk)
            if pj is not None:
                _emit_proj_rope(nc, tc, env, t, pj, part=0)
            pk = _emit_attn_scores(nc, tc, env, i, 1)
            pop_fill(npop)
            _emit_attn_rest(nc, tc, env, i, 1, pk)
            if pj is not None:
                _emit_proj_rope(nc, tc, env, t, pj, part=1)
            for dc in range(D // 256):
                fillers.append(
                    lambda i=i, dc=dc: _emit_op_chain(nc, tc, env, i, dc))
        elif pj is not None:
            _emit_proj_rope(nc, tc, env, t, pj, part=0)
            _emit_proj_rope(nc, tc, env, t, pj, part=1)
    pop_fill(len(fillers))


def _stat_pair(nc, env, srcs, pair):
    """rr = exp(-0.5*ln(mean(x^2)+eps)) for a pair of DH-wide sources.
    Ln+Exp share an ACT function table with Square/Copy: no table reloads."""
    f32, Act = env["f32"], env["Act"]
    tpool, spool = env["tpool"], env["spool"]
    ss2 = spool.tile([P, 2], f32, tag=f"ss{pair}", name="ss2")
    for j in (0, 1):
        sq = tpool.tile([P, DH], f32, tag="sq", bufs=2)
        nc.scalar.activation(sq[:], srcs[j], Act.Square,
                             accum_out=ss2[:, j:j + 1])
    lg2 = spool.tile([P, 2], f32, tag=f"lg{pair}", name="lg2")
    nc.scalar.activation(lg2[:], ss2[:], Act.Ln,
                         bias=env["eps_sb"][:], scale=1.0 / DH)
    rr2 = spool.tile([P, 2], f32, tag=f"rr{pair}", name="rr2")
    nc.scalar.activation(rr2[:], lg2[:], Act.Exp, scale=-0.5)
    return rr2


def _emit_proj_q(nc, tc, env, tt):
    """q projection matmuls for token tile tt + PSUM->SBUF evacuation + rms
    statistics.  The kv half is emitted separately (inside the h0 softmax
    shadow) via _emit_proj_kv."""
    f32, md = env["f32"], env["md"]
    xT_sb = env["xT_sb"]
    tpool, psum = env["tpool"], env["psum"]

    q_ps = psum.tile([P, NH * DH], f32, tag="s", bufs=S_BUFS, name="q_ps")
    for d in range(DT):
        nc.tensor.matmul(q_ps[:], xT_sb[:, d, tt * P:(tt + 1) * P],
                         env["wq_sb"][:, d, :],
                         start=(d == 0), stop=(d == DT - 1))
    qsb = tpool.tile([P, NH * DH], md, tag="qsb", bufs=3)
    nc.scalar.copy(qsb[:], q_ps[:])
    rr_q = _stat_pair(nc, env, [qsb[:, 0:DH], qsb[:, DH:2 * DH]], 0)
    return dict(qsb=qsb, rr_q=rr_q)


def _emit_proj_kv(nc, tc, env, tt, pj):
    f32, md = env["f32"], env["md"]
    xT_sb = env["xT_sb"]
    tpool, psum = env["tpool"], env["psum"]

    kv_ps = psum.tile([P, 2 * DH], f32, tag="s", bufs=S_BUFS, name="kv_ps")
    for d in range(DT):
        nc.tensor.matmul(kv_ps[:], xT_sb[:, d, tt * P:(tt + 1) * P],
                         env["wkv_sb"][:, d, :],
                         start=(d == 0), stop=(d == DT - 1))
    kvsb = tpool.tile([P, 2 * DH], md, tag="kvsb", bufs=3)
    nc.scalar.copy(kvsb[:], kv_ps[:])
    rr_kv = _stat_pair(nc, env, [kvsb[:, 0:DH], kvsb[:, DH:2 * DH]], 1)
    pj["kvsb"] = kvsb
    pj["rr_kv"] = rr_kv


def _emit_proj_rope(nc, tc, env, tt, pj, part):
    """part 0: q heads norm+rope; part 1: k norm+rope and v norm.  rope+norm
    in ~5 DVE ops per head:
      u  = (src * rr) * c           (c = cos table)
      v  = (rot_half(src) * rr) * s   (signs folded into s; 2 half-ops)
      qr = u + v
    Head-major transposes run on the DMA engines (XBAR 128-block transpose),
    keeping the PE free."""
    f32, md = env["f32"], env["md"]
    Alu = env["Alu"]
    kT_sb, v_sb = env["kT_sb"], env["v_sb"]
    tpool = env["tpool"]

    hd = DH // 2
    compact = env["compact_rope"]
    whichs = (0, 1) if part == 0 else (2,)
    psum = env["psum"]
    ident = env["ident"]
    qr2 = tpool.tile([P, len(whichs) * DH], md,
                     tag="qr2" if part == 0 else "kr2", bufs=3)
    for slot, which in enumerate(whichs):
        if which < NH:
            src = pj["qsb"][:, which * DH:(which + 1) * DH]
            rr = pj["rr_q"][:, which:which + 1]
        else:
            src = pj["kvsb"][:, 0:DH]
            rr = pj["rr_kv"][:, 0:1]
        if compact:
            ch = env["rope_sb"][:, tt, 0:hd]
            sn = env["rope_sb"][:, tt, hd:2 * hd]
            sp = env["rope_sb"][:, tt, 2 * hd:3 * hd]
        else:
            if which < NH:
                cw = env["cqw_sb"][:, tt, :]
                sw = env["sqw_sb"][:, tt, :]
            else:
                cw = env["ckw_sb"][:, tt, :]
                sw = env["skw_sb"][:, tt, :]
        u = tpool.tile([P, DH], md, tag="qa", bufs=3)
        if compact:
            nc.vector.scalar_tensor_tensor(u[:, 0:hd], src[:, 0:hd], rr, ch,
                                           op0=Alu.mult, op1=Alu.mult)
            nc.vector.scalar_tensor_tensor(u[:, hd:DH], src[:, hd:DH], rr, ch,
                                           op0=Alu.mult, op1=Alu.mult)
        else:
            nc.vector.scalar_tensor_tensor(u[:], src, rr, cw,
                                           op0=Alu.mult, op1=Alu.mult)
        v = tpool.tile([P, DH], md, tag="t1", bufs=3)
        nc.vector.scalar_tensor_tensor(v[:, 0:hd], src[:, hd:DH], rr,
                                       sn if compact else sw[:, 0:hd],
                                       op0=Alu.mult, op1=Alu.mult)
        nc.vector.scalar_tensor_tensor(v[:, hd:DH], src[:, 0:hd], rr,
                                       sp if compact else sw[:, hd:DH],
                                       op0=Alu.mult, op1=Alu.mult)
        nc.vector.tensor_add(qr2[:, slot * DH:(slot + 1) * DH], u[:], v[:])
    if True:
        # PE transpose into head-major layout, then one batched DVE copy
        nblk = 2 * len(whichs)
        tp_ps = psum.tile([P, 512], md, tag="t", bufs=T_BUFS, name="tp_ps")
        for blk in range(nblk):
            nc.tensor.transpose(tp_ps[:, blk * P:(blk + 1) * P],
                                qr2[:, blk * P:(blk + 1) * P], ident[:])
        if part == 0:
            qT_t = tpool.tile([P, NH * 2, P], md, tag="qTt", bufs=5,
                              name="qT_t")
            nc.vector.tensor_copy(
                qT_t[:], tp_ps[:].rearrange("p (b q1) -> p b q1", b=4))
            env["qT_tiles"][tt] = qT_t
        else:
            nc.vector.tensor_copy(
                kT_sb[:, :, tt * P:(tt + 1) * P],
                tp_ps[:, 0:2 * P].rearrange("p (b q1) -> p b q1", b=2))
    if part == 1:
        # ---- v: rms-norm only, stays token-major (fp16 SBUF -> 4x mode)
        nc.vector.tensor_scalar_mul(v_sb[:, tt, :], pj["kvsb"][:, DH:2 * DH],
                                    pj["rr_kv"][:, 1:2])


def _emit_attn_scores(nc, tc, env, i, h):
    """Scores in 512-col PSUM chunks + per-chunk max + exp.  Returns the
    packet (pchunks, zs, W) consumed by _emit_attn_rest."""
    f32, md = env["f32"], env["md"]
    Alu, Act, X = env["Alu"], env["Act"], env["X"]
    kT_sb = env["kT_sb"]
    trif_sb = env["trif_sb"]
    ppool, spool, psum = env["ppool"], env["spool"], env["psum"]
    qT_t = env["qT_tiles"][i]

    nlive = i + 1
    W = (nlive * P + KC - 1) // KC
    mxs = spool.tile([P, 5], f32, tag="mxs", name="mxs")
    schunks = []
    for c in range(W):
        k0 = c * KC
        k1 = min((c + 1) * KC, nlive * P)
        wc = k1 - k0
        s = psum.tile([P, KC], f32, tag="s", bufs=S_BUFS, name="s")
        for dh in (0, 1):
            nc.tensor.matmul(
                s[:, 0:wc], qT_t[:, h * 2 + dh, :],
                kT_sb[:, dh, k0:k1],
                start=(dh == 0), stop=(dh == 1))
        if c == W - 1:
            # additive causal mask on the diagonal 128x128 block (DVE)
            nc.vector.tensor_tensor(s[:, wc - P:wc], s[:, wc - P:wc],
                                    trif_sb[:], op=Alu.add)
        nc.vector.tensor_reduce(mxs[:, c:c + 1], s[:, 0:wc], axis=X,
                                op=Alu.max, negate=True)
        schunks.append((s, wc))
    negm = spool.tile([P, 1], f32, tag="negm", name="negm")
    nc.vector.tensor_reduce(negm[:], mxs[:, 0:W], axis=X, op=Alu.min)

    zs = spool.tile([P, 4], f32, tag="zs", name="zs")
    pchunks = []
    for c, (s, wc) in enumerate(schunks):
        p = ppool.tile([P, KC], md, tag="p")
        nc.scalar.activation(p[:, 0:wc], s[:, 0:wc], Act.Exp,
                             bias=negm[:], accum_out=zs[:, c:c + 1])
        pchunks.append((p, wc))
    return (pchunks, zs, W)


def _emit_attn_rest(nc, tc, env, i, h, pk):
    f32, md = env["f32"], env["md"]
    Alu, X = env["Alu"], env["X"]
    ident, v_sb = env["ident"], env["v_sb"]
    ptpool, tpool, spool, psum = (env["ptpool"], env["tpool"], env["spool"],
                                  env["psum"])
    pchunks, zs, W = pk
    if h == 0:
        env["aT_tiles"][i] = tpool.tile([P, NH * 2, P], md, tag="aTt",
                                        bufs=3, name="aT_t")
    aT_t = env["aT_tiles"][i]

    nlive = i + 1
    a_ps = psum.tile([P, KC], f32, tag="ao", bufs=AO_BUFS, name="a_ps")
    gl = 0
    for (p, wc) in pchunks:
        nbl = wc // P
        trp = psum.tile([P, KC], md, tag="t", bufs=T_BUFS, name="trp")
        for j in range(nbl):
            nc.tensor.transpose(trp[:, j * P:(j + 1) * P],
                                p[:, j * P:(j + 1) * P], ident[:])
        pt = ptpool.tile([P, KC], md, tag="pt")
        nc.vector.tensor_copy(pt[:, 0:nbl * P], trp[:, 0:nbl * P])
        for j in range(nbl):
            nc.tensor.matmul(a_ps[:, 0:DH], pt[:, j * P:(j + 1) * P],
                             v_sb[:, gl, :],
                             start=(gl == 0), stop=(gl == nlive - 1))
            gl += 1

    # normalize + transpose to head-major aT
    z = spool.tile([P, 1], f32, tag="z", name="z")
    nc.vector.reduce_sum(z[:], zs[:, 0:W], axis=X)
    rz = spool.tile([P, 1], f32, tag="rz", name="rz")
    nc.vector.reciprocal(rz[:], z[:])
    if True:
        at = tpool.tile([P, DH], md, tag="at", bufs=3)
        nc.vector.tensor_scalar_mul(at[:], a_ps[:, 0:DH], rz[:])
        atp = psum.tile([P, KC], md, tag="t", bufs=T_BUFS, name="atp")
        for e in range(2):
            nc.tensor.transpose(atp[:, e * P:(e + 1) * P],
                                at[:, e * P:(e + 1) * P], ident[:])
        nc.vector.tensor_copy(
            aT_t[:, h * 2:h * 2 + 2, :],
            atp[:, 0:2 * P].rearrange("p (b q1) -> p b q1", b=2))


def _emit_op_chain(nc, tc, env, i, dc):
    f32 = env["f32"]
    wo_sb, out_d = env["wo_sb"], env["out_d"]
    out_md = env["out_md"]
    opool, psum = env["opool"], env["psum"]
    aT_t = env["aT_tiles"][i]

    OC = 256  # half-bank chains: finer-grained PE filler
    ET = NH * DH // P  # 4
    o_ps = psum.tile([P, OC], f32, tag="ao", bufs=AO_BUFS, name="o_ps")
    for e in range(ET):
        nc.tensor.matmul(
            o_ps[:], aT_t[:, e, :],
            wo_sb[:, e, dc * OC:(dc + 1) * OC],
            start=(e == 0), stop=(e == ET - 1))
    o_sb = opool.tile([P, OC], out_md, tag="o")
    nc.scalar.copy(o_sb[:], o_ps[:])
    nc.sync.dma_start(
        out_d[i * P:(i + 1) * P, dc * OC:(dc + 1) * OC], o_sb[:])


def _can_compact(inputs):
    """Compact rope path needs all-ones norm weights, batch-identical
    cos/sin, and identical cos/sin halves (true for the reference RoPE)."""
    cos = np.asarray(inputs["cos"], np.float32)
    sin = np.asarray(inputs["sin"], np.float32)
    qnw = np.asarray(inputs["q_norm_w"], np.float32)
    knw = np.asarray(inputs["k_norm_w"], np.float32)
    hd = DH // 2
    return (np.all(qnw == 1.0) and np.all(knw == 1.0)
            and all(np.array_equal(cos[0], cos[b]) for b in range(1, B))
            and all(np.array_equal(sin[0], sin[b]) for b in range(1, B))
            and np.array_equal(cos[0][:, :hd], cos[0][:, hd:])
            and np.array_equal(sin[0][:, :hd], sin[0][:, hd:]))


def _host_prep(inputs, compact_rope=None):
    """Build the 8 per-core input maps from full inputs."""
    x = np.asarray(inputs["hidden_states"], np.float32)
    cos = np.asarray(inputs["cos"], np.float32)
    sin = np.asarray(inputs["sin"], np.float32)
    wq = np.asarray(inputs["wq"], np.float32)
    wk = np.asarray(inputs["wk"], np.float32)
    wv = np.asarray(inputs["wv"], np.float32)
    wo = np.asarray(inputs["wo"], np.float32)
    qnw = np.asarray(inputs["q_norm_w"], np.float32)
    knw = np.asarray(inputs["k_norm_w"], np.float32)

    if compact_rope is None:
        compact_rope = _can_compact(inputs)
    md = _np_md()
    hd = DH // 2

    if compact_rope:
        ch = cos[0][:, 0:hd]
        sh = sin[0][:, 0:hd]
        rope3 = [np.ascontiguousarray(
            np.concatenate([ch, -sh, sh], axis=1)).astype(md)] * B
    else:
        # rope tables with norm weight and rotate-half signs folded in
        sign = np.concatenate([-np.ones(hd), np.ones(hd)]).astype(np.float32)

        def _rope_tabs(w):
            w_rot = np.concatenate([w[hd:], w[:hd]])
            cw = [np.ascontiguousarray(cos[b] * w[None, :]).astype(md)
                  for b in range(B)]
            sw = [np.ascontiguousarray(
                      sin[b] * (sign * w_rot)[None, :]).astype(md)
                  for b in range(B)]
            return cw, sw

        cqw, sqw = _rope_tabs(qnw)
        ckw, skw = _rope_tabs(knw)

    # additive lower-triangular mask for the diagonal 128x128 block (fp32)
    r = np.arange(P)[:, None]
    c = np.arange(P)[None, :]
    trimaskf = np.where(c <= r, 0.0, NEG).astype(np.float32)

    xT = [np.ascontiguousarray(x[b].T).astype(md) for b in range(B)]

    in_maps = []
    for cid in range(8):
        b = cid // 4
        j = cid % 4
        h0 = 2 * j
        g = j // 2
        wqT = np.ascontiguousarray(wq[h0 * DH:(h0 + 2) * DH, :].T).astype(md)
        wkvT = np.ascontiguousarray(
            np.concatenate([wk[g * DH:(g + 1) * DH, :],
                            wv[g * DH:(g + 1) * DH, :]], axis=0).T).astype(md)
        woT2 = np.ascontiguousarray(wo[:, h0 * DH:(h0 + 2) * DH].T).astype(md)
        def v2(a):
            return a.view(np.uint16) if a.dtype.itemsize == 2 else a
        im = {
            "xT": v2(xT[b]),
            "wqT": v2(wqT),
            "wkvT": v2(wkvT),
            "woT2": v2(woT2),
            "trimaskf": trimaskf,
            "ident": v2(np.eye(P, dtype=md)),
        }
        if compact_rope:
            im["rope3"] = v2(rope3[b])
        else:
            im["cqw"] = v2(cqw[b])
            im["sqw"] = v2(sqw[b])
            im["ckw"] = v2(ckw[b])
            im["skw"] = v2(skw[b])
        in_maps.append(im)
    return in_maps


def kernel(**inputs) -> np.ndarray:
    compact = _can_compact(inputs)
    key = ("nc", compact)
    if key not in _cache:
        _cache[key] = _build_program(compact_rope=compact)
    nc = _cache[key]
    _cache["nc"] = nc  # last-built program, for the test harness
    in_maps = _host_prep(inputs, compact_rope=compact)
    res = bass_utils.run_bass_kernel_spmd(
        nc, in_maps, core_ids=list(range(8)))
    _cache["last_result"] = res
    out = np.zeros((B, S, D), np.float32)
    for cid in range(8):
        part = res.results[cid]["out"]
        if part.dtype == np.uint16:
            part = part.view(ml_dtypes.bfloat16).astype(np.float32)
        out[cid // 4] += part
    return out



# revision 47
# speedup vs baseline: 1.0342x; 1.0076x over previous
"""TRN2 Bass kernel for nn_AttentionModel_46823733461774.

Gemma3n-style attention block: qkv projection, q/k/v RMS-norm, RoPE on q/k,
GQA causal attention (no scaling; q_norm replaces 1/sqrt(d)), output proj.

Shapes (hardcoded): B=2, S=2048, D=2048, H=8, KV=2, DH=256.

Sharding over 8 cores: core c -> batch b=c//4, q-heads {2j, 2j+1} (j=c%4),
kv-head j//2.  Each core computes the projections for its batch/heads
(token-major), norms+RoPE, causal attention for its 2 heads, and a partial
output projection attn_heads @ wo_slice^T.  Host sums the 4 partials per
batch.  cos/sin replicated.

All matmuls in fp16 (same PE throughput as bf16, 8x the mantissa accuracy);
softmax statistics and accumulations in fp32.

Single merged pipeline: projection tiles and attention tiles are emitted
interleaved (attention for token-tile i follows projection of tile i+3), so
every engine keeps independent work during the softmax dependency chains.
Scores are computed in 512-column PSUM chunks from a 4-bank ring; the causal
mask is added to the diagonal chunk on DVE; row maxes are reduced per-chunk
on DVE as each chunk's matmuls finish; exp runs per-chunk on ACT; prob
transposes + PV accumulate chunk-by-chunk; 256-wide output-proj matmul
chains are used as fine-grained PE filler inside the softmax shadows.

Input DMA is pipelined in consumption order on the SP queue (per-queue DMA
transfers serialize): first xT chunk in dt-halves + wq quarters so the
tile-0 projection starts ~10us in, then wkv, rope, remaining xT chunks, wo.

Further structure: x^T is SBUF-resident; q^T / attn^T live in small ring
buffers; rms rsqrt is computed as exp(-0.5*ln(x)) so every ACT function
(square/ln/exp/copy) lives in one activation table (no 1.3us table
reloads); when the norm weights are all-ones (the reference setup), a
single packed half-table [cos|-sin|+sin] serves q and k rope (1.5MB instead
of 4MB of DMA + SBUF), with a full-table fallback otherwise.
"""

import os
from collections import deque

import numpy as np
import ml_dtypes

import concourse.bass as bass
import concourse.mybir as mybir
import concourse.tile as tile
from concourse import bacc
from concourse import bass_utils

B, S, D = 2, 2048, 2048
H, KV, DH = 8, 2, 256
EPS = 1e-6
NEG = -30000.0   # additive causal mask (fp16-representable; exp() -> 0)
P = 128
TT = S // P      # 16 token tiles
DT = D // P      # 16 contraction tiles
NH = 2           # heads per core
KC = 512         # key chunk (scores free dim; one PSUM bank)

# matmul dtype mode: "f16" | "bf16" | "f32"
MODE = os.environ.get("KERNEL_MODE", "f16")
# repeat the body N times inside the NEFF (for wall-clock HW timing)
ITERS = int(os.environ.get("KERNEL_ITERS", "1"))

_cache = {}


def _np_md():
    if MODE == "bf16":
        return ml_dtypes.bfloat16
    if MODE == "f16":
        return np.float16
    return np.float32


def _bir_md():
    if MODE == "bf16":
        return mybir.dt.bfloat16
    if MODE == "f16":
        return mybir.dt.float16
    return mybir.dt.float32


def _build_program(compact_rope=True):
    f32 = mybir.dt.float32
    md = _bir_md()
    Alu = mybir.AluOpType
    Act = mybir.ActivationFunctionType
    X = mybir.AxisListType.X
    XY = mybir.AxisListType.XY

    nc = bacc.Bacc("TRN2", target_bir_lowering=False, debug=False, num_devices=8)

    # fp16 buffers hang at the PJRT/axon boundary -> declare 2-byte inputs
    # as uint16 and bitcast to the matmul dtype on the DRAM APs.
    io2 = mybir.dt.uint16 if mybir.dt.size(md) == 2 else md
    def _in2(name, shape):
        ap = nc.dram_tensor(name, shape, io2, kind="ExternalInput").ap()
        return ap.bitcast(md) if io2 != md else ap
    xT_d = _in2("xT", [D, S])
    wqT_d = _in2("wqT", [D, NH * DH])
    wkvT_d = _in2("wkvT", [D, 2 * DH])
    woT2_d = _in2("woT2", [NH * DH, D])
    if compact_rope:
        # norm weights are all-ones and cos/sin halves are identical, so a
        # single packed table [S, 3*hd] = [cos_half | -sin_half | +sin_half]
        # serves q and k (1.5MB instead of 4MB of DMA + SBUF).
        rope3_d = _in2("rope3", [S, 3 * (DH // 2)])
    else:
        # rope tables with the norm weight and rotate-half signs folded in:
        # cw = cos*w ; sw[d<hd] = -sin[d]*w[d+hd], sw[d>=hd] = sin[d]*w[d-hd]
        cqw_d = _in2("cqw", [S, DH])
        sqw_d = _in2("sqw", [S, DH])
        ckw_d = _in2("ckw", [S, DH])
        skw_d = _in2("skw", [S, DH])
    trimaskf_d = nc.dram_tensor("trimaskf", [P, P], f32,
                                kind="ExternalInput").ap()
    ident_d = _in2("ident", [P, P])
    # output partials in bf16 (halves the out DMA; host sums in fp32).
    # 2-byte IO declared as uint16 like the inputs (PJRT boundary quirk).
    if io2 == md:  # f32 mode
        out_d = nc.dram_tensor("out", [S, D], f32, kind="ExternalOutput").ap()
        out_md = f32
    else:
        out_d = nc.dram_tensor("out", [S, D], mybir.dt.uint16,
                               kind="ExternalOutput").ap().bitcast(
                                   mybir.dt.bfloat16)
        out_md = mybir.dt.bfloat16

    with tile.TileContext(nc) as tc:
        with (
            tc.tile_pool(name="const", bufs=1) as cpool,
            tc.tile_pool(name="resid", bufs=1) as rpool,
            tc.tile_pool(name="pbuf", bufs=8) as ppool,
            tc.tile_pool(name="ptbuf", bufs=6) as ptpool,
            tc.tile_pool(name="tmp", bufs=10) as tpool,
            tc.tile_pool(name="stat", bufs=12) as spool,
            tc.tile_pool(name="obuf", bufs=3) as opool,
            tc.tile_pool(name="psum", bufs=1, space="PSUM") as psum,
        ):
            # ---- SBUF tiles for constants / weights / x ----
            wq_sb = cpool.tile([P, DT, NH * DH], md, tag="wq")
            wkv_sb = cpool.tile([P, DT, 2 * DH], md, tag="wkv")
            wo_sb = cpool.tile([P, NH * DH // P, D], md, tag="wo")
            xT_sb = cpool.tile([P, DT, S], md, tag="xT")
            hd = DH // 2
            if compact_rope:
                rope_sb = cpool.tile([P, TT, 3 * hd], md, tag="rope3")
            else:
                cqw_sb = cpool.tile([P, TT, DH], md, tag="cqw")
                sqw_sb = cpool.tile([P, TT, DH], md, tag="sqw")
                ckw_sb = cpool.tile([P, TT, DH], md, tag="ckw")
                skw_sb = cpool.tile([P, TT, DH], md, tag="skw")
            trif_sb = cpool.tile([P, P], f32, tag="trif")
            ident = cpool.tile([P, P], md, tag="ident")
            eps_sb = cpool.tile([P, 1], f32, tag="eps")

            # ---- pipelined input DMA, in consumption order ----
            # xT in 256-token chunks (512B contiguous runs, full DMA speed);
            # weights in halves so the first proj matmuls can start early;
            # rope tables chunked alongside the x tiles they feed; wo last
            # (first consumer is the tile-0 output chain, ~4 tiles in).
            XC = 256
            NXC = S // XC

            def load_xt(ci):
                nc.sync.dma_start(
                    xT_sb[:, :, ci * XC:(ci + 1) * XC],
                    xT_d[:, ci * XC:(ci + 1) * XC].rearrange(
                        "(dt p) t -> p dt t", p=P))

            def load_rope(ci):
                if compact_rope:
                    nc.sync.dma_start(
                        rope_sb[:, 2 * ci:2 * ci + 2, :],
                        rope3_d[ci * XC:(ci + 1) * XC, :].rearrange(
                            "(tt p) d1 -> p tt d1", p=P))
                else:
                    for sb, dr in ((cqw_sb, cqw_d), (sqw_sb, sqw_d),
                                   (ckw_sb, ckw_d), (skw_sb, skw_d)):
                        nc.sync.dma_start(
                            sb[:, 2 * ci:2 * ci + 2, :],
                            dr[ci * XC:(ci + 1) * XC, :].rearrange(
                                "(tt p) d1 -> p tt d1", p=P))

            # first xT chunk in dt-halves interleaved with wq quarters, so
            # the tile-0 q-projection chain starts as soon as the first
            # 0.5MB pieces land instead of after 2.5MB
            QDT = DT // 4
            def load_wq_q(qi):
                nc.sync.dma_start(
                    wq_sb[:, qi * QDT:(qi + 1) * QDT, :],
                    wqT_d[qi * D // 4:(qi + 1) * D // 4, :].rearrange(
                        "(dt p) e -> p dt e", p=P))

            nc.sync.dma_start(
                xT_sb[:, 0:DT // 2, 0:XC],
                xT_d[0:D // 2, 0:XC].rearrange("(dt p) t -> p dt t", p=P))
            load_wq_q(0)
            load_wq_q(1)
            nc.sync.dma_start(
                xT_sb[:, DT // 2:DT, 0:XC],
                xT_d[D // 2:D, 0:XC].rearrange("(dt p) t -> p dt t", p=P))
            def load_wkv_q(qi):
                nc.sync.dma_start(
                    wkv_sb[:, qi * QDT:(qi + 1) * QDT, :],
                    wkvT_d[qi * D // 4:(qi + 1) * D // 4, :].rearrange(
                        "(dt p) e -> p dt e", p=P))

            load_wq_q(2)
            load_wkv_q(0)
            load_wq_q(3)
            for qi in range(1, 4):
                load_wkv_q(qi)
            nc.sync.dma_start(trif_sb[:], trimaskf_d)
            nc.sync.dma_start(ident[:], ident_d)
            nc.gpsimd.memset(eps_sb[:], EPS)
            load_rope(0)
            for ci in range(1, 4):
                load_xt(ci)
                load_rope(ci)
            nc.sync.dma_start(
                wo_sb[:], woT2_d.rearrange("(et p) d1 -> p et d1", p=P))
            for ci in range(4, NXC):
                load_xt(ci)
                load_rope(ci)

            # ---- persistent activations ----
            kT_sb = rpool.tile([P, 2, S], md, tag="kT")
            v_sb = rpool.tile([P, TT, DH], md, tag="v")        # token-major

            env = dict(
                f32=f32, md=md, Alu=Alu, Act=Act, X=X, XY=XY,
                wq_sb=wq_sb, wkv_sb=wkv_sb, wo_sb=wo_sb, trif_sb=trif_sb,
                ident=ident, eps_sb=eps_sb, kT_sb=kT_sb,
                v_sb=v_sb, xT_sb=xT_sb, out_d=out_d, out_md=out_md,
                ppool=ppool, ptpool=ptpool, tpool=tpool,
                spool=spool, opool=opool, psum=psum,
                qT_tiles={}, aT_tiles={},
            )
            if compact_rope:
                env["rope_sb"] = rope_sb
            else:
                env.update(cqw_sb=cqw_sb, sqw_sb=sqw_sb,
                           ckw_sb=ckw_sb, skw_sb=skw_sb)
            env["compact_rope"] = compact_rope
            import contextlib
            unroll = int(os.environ.get("KERNEL_UNROLL", "1"))
            loop_ctx = (tc.For_i(0, ITERS, 1) if ITERS > 1
                        else contextlib.nullcontext())
            with loop_ctx:
                for _ in range(unroll):
                    _emit_body(nc, tc, env)

    # Activation-table pre-placement: the stock pass greedily maps each
    # activation to the FIRST act_info.json set containing its function,
    # which ping-pongs between the exp table and the ln table (1.3us reload
    # each).  Pre-place loads with our functions masked out of every set
    # before natural_log_exp_and_others, so everything first-matches that
    # one set (it contains exp+ln+square+copy+identity) and a single load
    # suffices.  Set ids keep their original act_info.json indices.
    from concourse.hw_specs import get_activation_tables
    import bass_rust as _br
    Act = mybir.ActivationFunctionType
    tables = list(get_activation_tables(nc.m.arch).items())
    target = next(idx for idx, (n, s) in enumerate(tables)
                  if n == "natural_log_exp_and_others")
    ours = {Act.Square, Act.Ln, Act.Exp, Act.Copy, Act.Identity}
    tables = [(n, (s - ours) if idx < target else s)
              for idx, (n, s) in enumerate(tables)]
    _br.insert_act_table_loads(nc, tables)

    nc.compile()
    return nc


# PSUM bank budget (8 banks): score-chunk ring "s" x4, transpose landing
# "t" x2, PV-accumulator / out-proj chain ring "ao" x2.
S_BUFS = 4
T_BUFS = 2
AO_BUFS = 2
PRE = 3  # attention for tile i is emitted after projection of tile i+PRE


def _emit_body(nc, tc, env):
    fillers = deque()

    def pop_fill(n):
        for _ in range(min(n, len(fillers))):
            fillers.popleft()()

    for t in range(TT + PRE):
        i = t - PRE
        pj = _emit_proj_q(nc, tc, env, t) if t < TT else None
        if pj is not None:
            _emit_proj_kv(nc, tc, env, t, pj)
        if 0 <= i < TT:
            # i-scaled filler: bank surplus chains early, spend them on the
            # longer softmax shadows of late (wide-W) tiles
            npop = 3 if i < 8 else (4 if i < 13 else 6)
            pk = _emit_attn_scores(nc, tc, env, i, 0)
            pop_fill(npop)
            _emit_attn_rest(nc, tc, env, i, 0, pk)
            if pj is not None:
                _emit_proj_rope(nc, tc, env, t, pj, part=0)
            pk = _emit_attn_scores(nc, tc, env, i, 1)
            pop_fill(npop)
            _emit_attn_rest(nc, tc, env, i, 1, pk)
            if pj is not None:
                _emit_proj_rope(nc, tc, env, t, pj, part=1)
            for dc in range(D // 256):
                fillers.append(
                    lambda i=i, dc=dc: _emit_op_chain(nc, tc, env, i, dc))
        elif pj is not None:
            _emit_proj_rope(nc, tc, env, t, pj, part=0)
            _emit_proj_rope(nc, tc, env, t, pj, part=1)
    pop_fill(len(fillers))


def _stat_pair(nc, env, srcs, pair):
    """rr = exp(-0.5*ln(mean(x^2)+eps)) for a pair of DH-wide sources.
    Ln+Exp share an ACT function table with Square/Copy: no table reloads."""
    f32, Act = env["f32"], env["Act"]
    tpool, spool = env["tpool"], env["spool"]
    ss2 = spool.tile([P, 2], f32, tag=f"ss{pair}", name="ss2")
    for j in (0, 1):
        sq = tpool.tile([P, DH], f32, tag="sq", bufs=2)
        nc.scalar.activation(sq[:], srcs[j], Act.Square,
                             accum_out=ss2[:, j:j + 1])
    lg2 = spool.tile([P, 2], f32, tag=f"lg{pair}", name="lg2")
    nc.scalar.activation(lg2[:], ss2[:], Act.Ln,
                         bias=env["eps_sb"][:], scale=1.0 / DH)
    rr2 = spool.tile([P, 2], f32, tag=f"rr{pair}", name="rr2")
    nc.scalar.activation(rr2[:], lg2[:], Act.Exp, scale=-0.5)
    return rr2


def _emit_proj_q(nc, tc, env, tt):
    """q projection matmuls for token tile tt + PSUM->SBUF evacuation + rms
    statistics.  The kv half is emitted separately (inside the h0 softmax
    shadow) via _emit_proj_kv."""
    f32, md = env["f32"], env["md"]
    xT_sb = env["xT_sb"]
    tpool, psum = env["tpool"], env["psum"]

    q_ps = psum.tile([P, NH * DH], f32, tag="s", bufs=S_BUFS, name="q_ps")
    for d in range(DT):
        nc.tensor.matmul(q_ps[:], xT_sb[:, d, tt * P:(tt + 1) * P],
                         env["wq_sb"][:, d, :],
                         start=(d == 0), stop=(d == DT - 1))
    qsb = tpool.tile([P, NH * DH], md, tag="qsb", bufs=3)
    nc.scalar.copy(qsb[:], q_ps[:])
    rr_q = _stat_pair(nc, env, [qsb[:, 0:DH], qsb[:, DH:2 * DH]], 0)
    return dict(qsb=qsb, rr_q=rr_q)


def _emit_proj_kv(nc, tc, env, tt, pj):
    f32, md = env["f32"], env["md"]
    xT_sb = env["xT_sb"]
    tpool, psum = env["tpool"], env["psum"]

    kv_ps = psum.tile([P, 2 * DH], f32, tag="s", bufs=S_BUFS, name="kv_ps")
    for d in range(DT):
        nc.tensor.matmul(kv_ps[:], xT_sb[:, d, tt * P:(tt + 1) * P],
                         env["wkv_sb"][:, d, :],
                         start=(d == 0), stop=(d == DT - 1))
    kvsb = tpool.tile([P, 2 * DH], md, tag="kvsb", bufs=3)
    nc.scalar.copy(kvsb[:], kv_ps[:])
    rr_kv = _stat_pair(nc, env, [kvsb[:, 0:DH], kvsb[:, DH:2 * DH]], 1)
    pj["kvsb"] = kvsb
    pj["rr_kv"] = rr_kv


def _emit_proj_rope(nc, tc, env, tt, pj, part):
    """part 0: q heads norm+rope; part 1: k norm+rope and v norm.  rope+norm
    in ~5 DVE ops per head:
      u  = (src * rr) * c           (c = cos table)
      v  = (rot_half(src) * rr) * s   (signs folded into s; 2 half-ops)
      qr = u + v
    Head-major transposes run on the DMA engines (XBAR 128-block transpose),
    keeping the PE free."""
    f32, md = env["f32"], env["md"]
    Alu = env["Alu"]
    kT_sb, v_sb = env["kT_sb"], env["v_sb"]
    tpool = env["tpool"]

    hd = DH // 2
    compact = env["compact_rope"]
    whichs = (0, 1) if part == 0 else (2,)
    psum = env["psum"]
    ident = env["ident"]
    qr2 = tpool.tile([P, len(whichs) * DH], md,
                     tag="qr2" if part == 0 else "kr2", bufs=3)
    for slot, which in enumerate(whichs):
        if which < NH:
            src = pj["qsb"][:, which * DH:(which + 1) * DH]
            rr = pj["rr_q"][:, which:which + 1]
        else:
            src = pj["kvsb"][:, 0:DH]
            rr = pj["rr_kv"][:, 0:1]
        if compact:
            ch = env["rope_sb"][:, tt, 0:hd]
            sn = env["rope_sb"][:, tt, hd:2 * hd]
            sp = env["rope_sb"][:, tt, 2 * hd:3 * hd]
        else:
            if which < NH:
                cw = env["cqw_sb"][:, tt, :]
                sw = env["sqw_sb"][:, tt, :]
            else:
                cw = env["ckw_sb"][:, tt, :]
                sw = env["skw_sb"][:, tt, :]
        u = tpool.tile([P, DH], md, tag="qa", bufs=3)
        if compact:
            nc.vector.scalar_tensor_tensor(u[:, 0:hd], src[:, 0:hd], rr, ch,
                                           op0=Alu.mult, op1=Alu.mult)
            nc.vector.scalar_tensor_tensor(u[:, hd:DH], src[:, hd:DH], rr, ch,
                                           op0=Alu.mult, op1=Alu.mult)
        else:
            nc.vector.scalar_tensor_tensor(u[:], src, rr, cw,
                                           op0=Alu.mult, op1=Alu.mult)
        v = tpool.tile([P, DH], md, tag="t1", bufs=3)
        nc.vector.scalar_tensor_tensor(v[:, 0:hd], src[:, hd:DH], rr,
                                       sn if compact else sw[:, 0:hd],
                                       op0=Alu.mult, op1=Alu.mult)
        nc.vector.scalar_tensor_tensor(v[:, hd:DH], src[:, 0:hd], rr,
                                       sp if compact else sw[:, hd:DH],
                                       op0=Alu.mult, op1=Alu.mult)
        nc.vector.tensor_add(qr2[:, slot * DH:(slot + 1) * DH], u[:], v[:])
    if True:
        # PE transpose into head-major layout, then one batched DVE copy
        nblk = 2 * len(whichs)
        tp_ps = psum.tile([P, 512], md, tag="t", bufs=T_BUFS, name="tp_ps")
        for blk in range(nblk):
            nc.tensor.transpose(tp_ps[:, blk * P:(blk + 1) * P],
                                qr2[:, blk * P:(blk + 1) * P], ident[:])
        if part == 0:
            qT_t = tpool.tile([P, NH * 2, P], md, tag="qTt", bufs=5,
                              name="qT_t")
            nc.vector.tensor_copy(
                qT_t[:], tp_ps[:].rearrange("p (b q1) -> p b q1", b=4))
            env["qT_tiles"][tt] = qT_t
        else:
            nc.vector.tensor_copy(
                kT_sb[:, :, tt * P:(tt + 1) * P],
                tp_ps[:, 0:2 * P].rearrange("p (b q1) -> p b q1", b=2))
    if part == 1:
        # ---- v: rms-norm only, stays token-major (fp16 SBUF -> 4x mode)
        nc.vector.tensor_scalar_mul(v_sb[:, tt, :], pj["kvsb"][:, DH:2 * DH],
                                    pj["rr_kv"][:, 1:2])


def _emit_attn_scores(nc, tc, env, i, h):
    """Scores in 512-col PSUM chunks + per-chunk max + exp.  Returns the
    packet (pchunks, zs, W) consumed by _emit_attn_rest."""
    f32, md = env["f32"], env["md"]
    Alu, Act, X = env["Alu"], env["Act"], env["X"]
    kT_sb = env["kT_sb"]
    trif_sb = env["trif_sb"]
    ppool, spool, psum = env["ppool"], env["spool"], env["psum"]
    qT_t = env["qT_tiles"][i]

    nlive = i + 1
    W = (nlive * P + KC - 1) // KC
    mxs = spool.tile([P, 5], f32, tag="mxs", name="mxs")
    schunks = []
    for c in range(W):
        k0 = c * KC
        k1 = min((c + 1) * KC, nlive * P)
        wc = k1 - k0
        s = psum.tile([P, KC], f32, tag="s", bufs=S_BUFS, name="s")
        for dh in (0, 1):
            nc.tensor.matmul(
                s[:, 0:wc], qT_t[:, h * 2 + dh, :],
                kT_sb[:, dh, k0:k1],
                start=(dh == 0), stop=(dh == 1))
        if c == W - 1:
            # additive causal mask on the diagonal 128x128 block (DVE)
            nc.vector.tensor_tensor(s[:, wc - P:wc], s[:, wc - P:wc],
                                    trif_sb[:], op=Alu.add)
        nc.vector.tensor_reduce(mxs[:, c:c + 1], s[:, 0:wc], axis=X,
                                op=Alu.max, negate=True)
        schunks.append((s, wc))
    negm = spool.tile([P, 1], f32, tag="negm", name="negm")
    nc.vector.tensor_reduce(negm[:], mxs[:, 0:W], axis=X, op=Alu.min)

    zs = spool.tile([P, 4], f32, tag="zs", name="zs")
    pchunks = []
    for c, (s, wc) in enumerate(schunks):
        p = ppool.tile([P, KC], md, tag="p")
        nc.scalar.activation(p[:, 0:wc], s[:, 0:wc], Act.Exp,
                             bias=negm[:], accum_out=zs[:, c:c + 1])
        pchunks.append((p, wc))
    return (pchunks, zs, W)


def _emit_attn_rest(nc, tc, env, i, h, pk):
    f32, md = env["f32"], env["md"]
    Alu, X = env["Alu"], env["X"]
    ident, v_sb = env["ident"], env["v_sb"]
    ptpool, tpool, spool, psum = (env["ptpool"], env["tpool"], env["spool"],
                                  env["psum"])
    pchunks, zs, W = pk
    if h == 0:
        env["aT_tiles"][i] = tpool.tile([P, NH * 2, P], md, tag="aTt",
                                        bufs=3, name="aT_t")
    aT_t = env["aT_tiles"][i]

    nlive = i + 1
    a_ps = psum.tile([P, KC], f32, tag="ao", bufs=AO_BUFS, name="a_ps")
    gl = 0
    for (p, wc) in pchunks:
        nbl = wc // P
        trp = psum.tile([P, KC], md, tag="t", bufs=T_BUFS, name="trp")
        for j in range(nbl):
            nc.tensor.transpose(trp[:, j * P:(j + 1) * P],
                                p[:, j * P:(j + 1) * P], ident[:])
        pt = ptpool.tile([P, KC], md, tag="pt")
        nc.vector.tensor_copy(pt[:, 0:nbl * P], trp[:, 0:nbl * P])
        for j in range(nbl):
            nc.tensor.matmul(a_ps[:, 0:DH], pt[:, j * P:(j + 1) * P],
                             v_sb[:, gl, :],
                             start=(gl == 0), stop=(gl == nlive - 1))
            gl += 1

    # normalize + transpose to head-major aT
    z = spool.tile([P, 1], f32, tag="z", name="z")
    nc.vector.reduce_sum(z[:], zs[:, 0:W], axis=X)
    rz = spool.tile([P, 1], f32, tag="rz", name="rz")
    nc.vector.reciprocal(rz[:], z[:])
    if True:
        at = tpool.tile([P, DH], md, tag="at", bufs=3)
        nc.vector.tensor_scalar_mul(at[:], a_ps[:, 0:DH], rz[:])
        atp = psum.tile([P, KC], md, tag="t", bufs=T_BUFS, name="atp")
        for e in range(2):
            nc.tensor.transpose(atp[:, e * P:(e + 1) * P],
                                at[:, e * P:(e + 1) * P], ident[:])
        nc.vector.tensor_copy(
            aT_t[:, h * 2:h * 2 + 2, :],
            atp[:, 0:2 * P].rearrange("p (b q1) -> p b q1", b=2))


def _emit_op_chain(nc, tc, env, i, dc):
    f32 = env["f32"]
    wo_sb, out_d = env["wo_sb"], env["out_d"]
    out_md = env["out_md"]
    opool, psum = env["opool"], env["psum"]
    aT_t = env["aT_tiles"][i]

    OC = 256  # half-bank chains: finer-grained PE filler
    ET = NH * DH // P  # 4
    o_ps = psum.tile([P, OC], f32, tag="ao", bufs=AO_BUFS, name="o_ps")
    for e in range(ET):
        nc.tensor.matmul(
            o_ps[:], aT_t[:, e, :],
            wo_sb[:, e, dc * OC:(dc + 1) * OC],
            start=(e == 0), stop=(e == ET - 1))
    o_sb = opool.tile([P, OC], out_md, tag="o")
    nc.scalar.copy(o_sb[:], o_ps[:])
    nc.sync.dma_start(
        out_d[i * P:(i + 1) * P, dc * OC:(dc + 1) * OC], o_sb[:])


def _can_compact(inputs):
    """Compact rope path needs all-ones norm weights, batch-identical
    cos/sin, and identical cos/sin halves (true for the reference RoPE)."""
    cos = np.asarray(inputs["cos"], np.float32)
    sin = np.asarray(inputs["sin"], np.float32)
    qnw = np.asarray(inputs["q_norm_w"], np.float32)
    knw = np.asarray(inputs["k_norm_w"], np.float32)
    hd = DH // 2
    return (np.all(qnw == 1.0) and np.all(knw == 1.0)
            and all(np.array_equal(cos[0], cos[b]) for b in range(1, B))
            and all(np.array_equal(sin[0], sin[b]) for b in range(1, B))
            and np.array_equal(cos[0][:, :hd], cos[0][:, hd:])
            and np.array_equal(sin[0][:, :hd], sin[0][:, hd:]))


def _host_prep(inputs, compact_rope=None):
    """Build the 8 per-core input maps from full inputs."""
    x = np.asarray(inputs["hidden_states"], np.float32)
    cos = np.asarray(inputs["cos"], np.float32)
    sin = np.asarray(inputs["sin"], np.float32)
    wq = np.asarray(inputs["wq"], np.float32)
    wk = np.asarray(inputs["wk"], np.float32)
    wv = np.asarray(inputs["wv"], np.float32)
    wo = np.asarray(inputs["wo"], np.float32)
    qnw = np.asarray(inputs["q_norm_w"], np.float32)
    knw = np.asarray(inputs["k_norm_w"], np.float32)

    if compact_rope is None:
        compact_rope = _can_compact(inputs)
    md = _np_md()
    hd = DH // 2

    if compact_rope:
        ch = cos[0][:, 0:hd]
        sh = sin[0][:, 0:hd]
        rope3 = [np.ascontiguousarray(
            np.concatenate([ch, -sh, sh], axis=1)).astype(md)] * B
    else:
        # rope tables with norm weight and rotate-half signs folded in
        sign = np.concatenate([-np.ones(hd), np.ones(hd)]).astype(np.float32)

        def _rope_tabs(w):
            w_rot = np.concatenate([w[hd:], w[:hd]])
            cw = [np.ascontiguousarray(cos[b] * w[None, :]).astype(md)
                  for b in range(B)]
            sw = [np.ascontiguousarray(
                      sin[b] * (sign * w_rot)[None, :]).astype(md)
                  for b in range(B)]
            return cw, sw

        cqw, sqw = _rope_tabs(qnw)
        ckw, skw = _rope_tabs(knw)

    # additive lower-triangular mask for the diagonal 128x128 block (fp32)
    r = np.arange(P)[:, None]
    c = np.arange(P)[None, :]
    trimaskf = np.where(c <= r, 0.0, NEG).astype(np.float32)

    xT = [np.ascontiguousarray(x[b].T).astype(md) for b in range(B)]

    in_maps = []
    for cid in range(8):
        b = cid // 4
        j = cid % 4
        h0 = 2 * j
        g = j // 2
        wqT = np.ascontiguousarray(wq[h0 * DH:(h0 + 2) * DH, :].T).astype(md)
        wkvT = np.ascontiguousarray(
            np.concatenate([wk[g * DH:(g + 1) * DH, :],
                            wv[g * DH:(g + 1) * DH, :]], axis=0).T).astype(md)
        woT2 = np.ascontiguousarray(wo[:, h0 * DH:(h0 + 2) * DH].T).astype(md)
        def v2(a):
            return a.view(np.uint16) if a.dtype.itemsize == 2 else a
        im = {
            "xT": v2(xT[b]),
            "wqT": v2(wqT),
            "wkvT": v2(wkvT),
            "woT2": v2(woT2),
            "trimaskf": trimaskf,
            "ident": v2(np.eye(P, dtype=md)),
        }
        if compact_rope:
            im["rope3"] = v2(rope3[b])
        else:
            im["cqw"] = v2(cqw[b])
            im["sqw"] = v2(sqw[b])
            im["ckw"] = v2(ckw[b])
            im["skw"] = v2(skw[b])
        in_maps.append(im)
    return in_maps


def kernel(**inputs) -> np.ndarray:
    compact = _can_compact(inputs)
    key = ("nc", compact)
    if key not in _cache:
        _cache[key] = _build_program(compact_rope=compact)
    nc = _cache[key]
    _cache["nc"] = nc  # last-built program, for the test harness
    in_maps = _host_prep(inputs, compact_rope=compact)
    res = bass_utils.run_bass_kernel_spmd(
        nc, in_maps, core_ids=list(range(8)))
    _cache["last_result"] = res
    out = np.zeros((B, S, D), np.float32)
    for cid in range(8):
        part = res.results[cid]["out"]
        if part.dtype == np.uint16:
            part = part.view(ml_dtypes.bfloat16).astype(np.float32)
        out[cid // 4] += part
    return out



# revision 48
# speedup vs baseline: 1.0357x; 1.0015x over previous
"""TRN2 Bass kernel for nn_AttentionModel_46823733461774.

Gemma3n-style attention block: qkv projection, q/k/v RMS-norm, RoPE on q/k,
GQA causal attention (no scaling; q_norm replaces 1/sqrt(d)), output proj.

Shapes (hardcoded): B=2, S=2048, D=2048, H=8, KV=2, DH=256.

Sharding over 8 cores: core c -> batch b=c//4, q-heads {2j, 2j+1} (j=c%4),
kv-head j//2.  Each core computes the projections for its batch/heads
(token-major), norms+RoPE, causal attention for its 2 heads, and a partial
output projection attn_heads @ wo_slice^T.  Host sums the 4 partials per
batch.  cos/sin replicated.

All matmuls in fp16 (same PE throughput as bf16, 8x the mantissa accuracy);
softmax statistics and accumulations in fp32.

Single merged pipeline: projection tiles and attention tiles are emitted
interleaved (attention for token-tile i follows projection of tile i+3), so
every engine keeps independent work during the softmax dependency chains.
Scores are computed in 512-column PSUM chunks from a 4-bank ring; the causal
mask is added to the diagonal chunk on DVE; row maxes are reduced per-chunk
on DVE as each chunk's matmuls finish; exp runs per-chunk on ACT; prob
transposes + PV accumulate chunk-by-chunk; 256-wide output-proj matmul
chains are used as fine-grained PE filler inside the softmax shadows.

Input DMA is pipelined in consumption order on the SP queue (per-queue DMA
transfers serialize): first xT chunk in dt-halves + wq quarters so the
tile-0 projection starts ~10us in, then wkv, rope, remaining xT chunks, wo.

Further structure: x^T is SBUF-resident; q^T / attn^T live in small ring
buffers; rms rsqrt is computed as exp(-0.5*ln(x)) so every ACT function
(square/ln/exp/copy) lives in one activation table (no 1.3us table
reloads); when the norm weights are all-ones (the reference setup), a
single packed half-table [cos|-sin|+sin] serves q and k rope (1.5MB instead
of 4MB of DMA + SBUF), with a full-table fallback otherwise.
"""

import os
from collections import deque

import numpy as np
import ml_dtypes

import concourse.bass as bass
import concourse.mybir as mybir
import concourse.tile as tile
from concourse import bacc
from concourse import bass_utils

B, S, D = 2, 2048, 2048
H, KV, DH = 8, 2, 256
EPS = 1e-6
NEG = -30000.0   # additive causal mask (fp16-representable; exp() -> 0)
P = 128
TT = S // P      # 16 token tiles
DT = D // P      # 16 contraction tiles
NH = 2           # heads per core
KC = 512         # key chunk (scores free dim; one PSUM bank)

# matmul dtype mode: "f16" | "bf16" | "f32"
MODE = os.environ.get("KERNEL_MODE", "f16")
# repeat the body N times inside the NEFF (for wall-clock HW timing)
ITERS = int(os.environ.get("KERNEL_ITERS", "1"))

_cache = {}


def _np_md():
    if MODE == "bf16":
        return ml_dtypes.bfloat16
    if MODE == "f16":
        return np.float16
    return np.float32


def _bir_md():
    if MODE == "bf16":
        return mybir.dt.bfloat16
    if MODE == "f16":
        return mybir.dt.float16
    return mybir.dt.float32


def _build_program(compact_rope=True):
    f32 = mybir.dt.float32
    md = _bir_md()
    Alu = mybir.AluOpType
    Act = mybir.ActivationFunctionType
    X = mybir.AxisListType.X
    XY = mybir.AxisListType.XY

    nc = bacc.Bacc("TRN2", target_bir_lowering=False, debug=False, num_devices=8)

    # fp16 buffers hang at the PJRT/axon boundary -> declare 2-byte inputs
    # as uint16 and bitcast to the matmul dtype on the DRAM APs.
    io2 = mybir.dt.uint16 if mybir.dt.size(md) == 2 else md
    def _in2(name, shape):
        ap = nc.dram_tensor(name, shape, io2, kind="ExternalInput").ap()
        return ap.bitcast(md) if io2 != md else ap
    xT_d = _in2("xT", [D, S])
    wqT_d = _in2("wqT", [D, NH * DH])
    wkvT_d = _in2("wkvT", [D, 2 * DH])
    woT2_d = _in2("woT2", [NH * DH, D])
    if compact_rope:
        # norm weights are all-ones and cos/sin halves are identical, so a
        # single packed table [S, 3*hd] = [cos_half | -sin_half | +sin_half]
        # serves q and k (1.5MB instead of 4MB of DMA + SBUF).
        rope3_d = _in2("rope3", [S, 3 * (DH // 2)])
    else:
        # rope tables with the norm weight and rotate-half signs folded in:
        # cw = cos*w ; sw[d<hd] = -sin[d]*w[d+hd], sw[d>=hd] = sin[d]*w[d-hd]
        cqw_d = _in2("cqw", [S, DH])
        sqw_d = _in2("sqw", [S, DH])
        ckw_d = _in2("ckw", [S, DH])
        skw_d = _in2("skw", [S, DH])
    trimaskf_d = nc.dram_tensor("trimaskf", [P, P], f32,
                                kind="ExternalInput").ap()
    ident_d = _in2("ident", [P, P])
    # output partials in bf16 (halves the out DMA; host sums in fp32).
    # 2-byte IO declared as uint16 like the inputs (PJRT boundary quirk).
    if io2 == md:  # f32 mode
        out_d = nc.dram_tensor("out", [S, D], f32, kind="ExternalOutput").ap()
        out_md = f32
    else:
        out_d = nc.dram_tensor("out", [S, D], mybir.dt.uint16,
                               kind="ExternalOutput").ap().bitcast(
                                   mybir.dt.bfloat16)
        out_md = mybir.dt.bfloat16

    with tile.TileContext(nc) as tc:
        with (
            tc.tile_pool(name="const", bufs=1) as cpool,
            tc.tile_pool(name="resid", bufs=1) as rpool,
            tc.tile_pool(name="pbuf", bufs=8) as ppool,
            tc.tile_pool(name="ptbuf", bufs=6) as ptpool,
            tc.tile_pool(name="tmp", bufs=10) as tpool,
            tc.tile_pool(name="stat", bufs=12) as spool,
            tc.tile_pool(name="obuf", bufs=3) as opool,
            tc.tile_pool(name="psum", bufs=1, space="PSUM") as psum,
        ):
            # ---- SBUF tiles for constants / weights / x ----
            wq_sb = cpool.tile([P, DT, NH * DH], md, tag="wq")
            wkv_sb = cpool.tile([P, DT, 2 * DH], md, tag="wkv")
            wo_sb = cpool.tile([P, NH * DH // P, D], md, tag="wo")
            xT_sb = cpool.tile([P, DT, S], md, tag="xT")
            hd = DH // 2
            if compact_rope:
                rope_sb = cpool.tile([P, TT, 3 * hd], md, tag="rope3")
            else:
                cqw_sb = cpool.tile([P, TT, DH], md, tag="cqw")
                sqw_sb = cpool.tile([P, TT, DH], md, tag="sqw")
                ckw_sb = cpool.tile([P, TT, DH], md, tag="ckw")
                skw_sb = cpool.tile([P, TT, DH], md, tag="skw")
            trif_sb = cpool.tile([P, P], f32, tag="trif")
            ident = cpool.tile([P, P], md, tag="ident")
            eps_sb = cpool.tile([P, 1], f32, tag="eps")

            # ---- pipelined input DMA, in consumption order ----
            # xT in 256-token chunks (512B contiguous runs, full DMA speed);
            # weights in halves so the first proj matmuls can start early;
            # rope tables chunked alongside the x tiles they feed; wo last
            # (first consumer is the tile-0 output chain, ~4 tiles in).
            XC = 256
            NXC = S // XC

            def load_xt(ci):
                nc.sync.dma_start(
                    xT_sb[:, :, ci * XC:(ci + 1) * XC],
                    xT_d[:, ci * XC:(ci + 1) * XC].rearrange(
                        "(dt p) t -> p dt t", p=P))

            def load_rope(ci):
                if compact_rope:
                    nc.sync.dma_start(
                        rope_sb[:, 2 * ci:2 * ci + 2, :],
                        rope3_d[ci * XC:(ci + 1) * XC, :].rearrange(
                            "(tt p) d1 -> p tt d1", p=P))
                else:
                    for sb, dr in ((cqw_sb, cqw_d), (sqw_sb, sqw_d),
                                   (ckw_sb, ckw_d), (skw_sb, skw_d)):
                        nc.sync.dma_start(
                            sb[:, 2 * ci:2 * ci + 2, :],
                            dr[ci * XC:(ci + 1) * XC, :].rearrange(
                                "(tt p) d1 -> p tt d1", p=P))

            # first xT chunk in dt-halves interleaved with wq quarters, so
            # the tile-0 q-projection chain starts as soon as the first
            # 0.5MB pieces land instead of after 2.5MB
            QDT = DT // 4
            def load_wq_q(qi):
                nc.sync.dma_start(
                    wq_sb[:, qi * QDT:(qi + 1) * QDT, :],
                    wqT_d[qi * D // 4:(qi + 1) * D // 4, :].rearrange(
                        "(dt p) e -> p dt e", p=P))

            nc.sync.dma_start(
                xT_sb[:, 0:DT // 2, 0:XC],
                xT_d[0:D // 2, 0:XC].rearrange("(dt p) t -> p dt t", p=P))
            load_wq_q(0)
            load_wq_q(1)
            nc.sync.dma_start(
                xT_sb[:, DT // 2:DT, 0:XC],
                xT_d[D // 2:D, 0:XC].rearrange("(dt p) t -> p dt t", p=P))
            def load_wkv_q(qi):
                nc.sync.dma_start(
                    wkv_sb[:, qi * QDT:(qi + 1) * QDT, :],
                    wkvT_d[qi * D // 4:(qi + 1) * D // 4, :].rearrange(
                        "(dt p) e -> p dt e", p=P))

            load_wq_q(2)
            load_wkv_q(0)
            load_wq_q(3)
            for qi in range(1, 4):
                load_wkv_q(qi)
            nc.sync.dma_start(trif_sb[:], trimaskf_d)
            nc.sync.dma_start(ident[:], ident_d)
            nc.gpsimd.memset(eps_sb[:], EPS)
            load_rope(0)
            for ci in range(1, 4):
                load_xt(ci)
                load_rope(ci)
            nc.sync.dma_start(
                wo_sb[:], woT2_d.rearrange("(et p) d1 -> p et d1", p=P))
            for ci in range(4, NXC):
                load_xt(ci)
                load_rope(ci)

            # ---- persistent activations ----
            kT_sb = rpool.tile([P, 2, S], md, tag="kT")
            v_sb = rpool.tile([P, TT, DH], md, tag="v")        # token-major

            env = dict(
                f32=f32, md=md, Alu=Alu, Act=Act, X=X, XY=XY,
                wq_sb=wq_sb, wkv_sb=wkv_sb, wo_sb=wo_sb, trif_sb=trif_sb,
                ident=ident, eps_sb=eps_sb, kT_sb=kT_sb,
                v_sb=v_sb, xT_sb=xT_sb, out_d=out_d, out_md=out_md,
                ppool=ppool, ptpool=ptpool, tpool=tpool,
                spool=spool, opool=opool, psum=psum,
                qT_tiles={}, aT_tiles={},
            )
            if compact_rope:
                env["rope_sb"] = rope_sb
            else:
                env.update(cqw_sb=cqw_sb, sqw_sb=sqw_sb,
                           ckw_sb=ckw_sb, skw_sb=skw_sb)
            env["compact_rope"] = compact_rope
            import contextlib
            unroll = int(os.environ.get("KERNEL_UNROLL", "1"))
            loop_ctx = (tc.For_i(0, ITERS, 1) if ITERS > 1
                        else contextlib.nullcontext())
            with loop_ctx:
                for _ in range(unroll):
                    _emit_body(nc, tc, env)

    # Activation-table pre-placement: the stock pass greedily maps each
    # activation to the FIRST act_info.json set containing its function,
    # which ping-pongs between the exp table and the ln table (1.3us reload
    # each).  Pre-place loads with our functions masked out of every set
    # before natural_log_exp_and_others, so everything first-matches that
    # one set (it contains exp+ln+square+copy+identity) and a single load
    # suffices.  Set ids keep their original act_info.json indices.
    from concourse.hw_specs import get_activation_tables
    import bass_rust as _br
    Act = mybir.ActivationFunctionType
    tables = list(get_activation_tables(nc.m.arch).items())
    target = next(idx for idx, (n, s) in enumerate(tables)
                  if n == "natural_log_exp_and_others")
    ours = {Act.Square, Act.Ln, Act.Exp, Act.Copy, Act.Identity}
    tables = [(n, (s - ours) if idx < target else s)
              for idx, (n, s) in enumerate(tables)]
    _br.insert_act_table_loads(nc, tables)

    nc.compile()
    return nc


# PSUM bank budget (8 banks): score-chunk ring "s" x4, transpose landing
# "t" x2, PV-accumulator / out-proj chain ring "ao" x2.
S_BUFS = 4
T_BUFS = 2
AO_BUFS = 2
PRE = 3  # attention for tile i is emitted after projection of tile i+PRE


def _emit_body(nc, tc, env):
    fillers = deque()

    def pop_fill(n):
        for _ in range(min(n, len(fillers))):
            fillers.popleft()()

    for t in range(TT + PRE):
        i = t - PRE
        pj = _emit_proj_q(nc, tc, env, t) if t < TT else None
        if pj is not None:
            _emit_proj_kv(nc, tc, env, t, pj)
        if 0 <= i < TT:
            # i-scaled filler: bank surplus chains early, spend them on the
            # longer softmax shadows of late (wide-W) tiles
            npop = 3 if i < 8 else (4 if i < 13 else 6)
            pk = _emit_attn_scores(nc, tc, env, i, 0)
            pop_fill(npop)
            _emit_attn_rest(nc, tc, env, i, 0, pk)
            if pj is not None:
                _emit_proj_rope(nc, tc, env, t, pj, part=0)
            pk = _emit_attn_scores(nc, tc, env, i, 1)
            pop_fill(npop)
            _emit_attn_rest(nc, tc, env, i, 1, pk)
            if pj is not None:
                _emit_proj_rope(nc, tc, env, t, pj, part=1)
            for dc in range(D // 256):
                fillers.append(
                    lambda i=i, dc=dc: _emit_op_chain(nc, tc, env, i, dc))
        elif pj is not None:
            _emit_proj_rope(nc, tc, env, t, pj, part=0)
            _emit_proj_rope(nc, tc, env, t, pj, part=1)
    pop_fill(len(fillers))


def _stat_pair(nc, env, srcs, pair):
    """rr = exp(-0.5*ln(mean(x^2)+eps)) for a pair of DH-wide sources.
    Ln+Exp share an ACT function table with Square/Copy: no table reloads."""
    f32, Act = env["f32"], env["Act"]
    tpool, spool = env["tpool"], env["spool"]
    ss2 = spool.tile([P, 2], f32, tag=f"ss{pair}", name="ss2")
    for j in (0, 1):
        sq = tpool.tile([P, DH], f32, tag="sq", bufs=2)
        nc.scalar.activation(sq[:], srcs[j], Act.Square,
                             accum_out=ss2[:, j:j + 1])
    lg2 = spool.tile([P, 2], f32, tag=f"lg{pair}", name="lg2")
    nc.scalar.activation(lg2[:], ss2[:], Act.Ln,
                         bias=env["eps_sb"][:], scale=1.0 / DH)
    rr2 = spool.tile([P, 2], f32, tag=f"rr{pair}", name="rr2")
    nc.scalar.activation(rr2[:], lg2[:], Act.Exp, scale=-0.5)
    return rr2


def _emit_proj_q(nc, tc, env, tt):
    """q projection matmuls for token tile tt + PSUM->SBUF evacuation + rms
    statistics.  The kv half is emitted separately (inside the h0 softmax
    shadow) via _emit_proj_kv."""
    f32, md = env["f32"], env["md"]
    xT_sb = env["xT_sb"]
    tpool, psum = env["tpool"], env["psum"]

    q_ps = psum.tile([P, NH * DH], f32, tag="s", bufs=S_BUFS, name="q_ps")
    for d in range(DT):
        nc.tensor.matmul(q_ps[:], xT_sb[:, d, tt * P:(tt + 1) * P],
                         env["wq_sb"][:, d, :],
                         start=(d == 0), stop=(d == DT - 1))
    qsb = tpool.tile([P, NH * DH], md, tag="qsb", bufs=3)
    nc.scalar.copy(qsb[:], q_ps[:])
    rr_q = _stat_pair(nc, env, [qsb[:, 0:DH], qsb[:, DH:2 * DH]], 0)
    return dict(qsb=qsb, rr_q=rr_q)


def _emit_proj_kv(nc, tc, env, tt, pj):
    f32, md = env["f32"], env["md"]
    xT_sb = env["xT_sb"]
    tpool, psum = env["tpool"], env["psum"]

    kv_ps = psum.tile([P, 2 * DH], f32, tag="s", bufs=S_BUFS, name="kv_ps")
    for d in range(DT):
        nc.tensor.matmul(kv_ps[:], xT_sb[:, d, tt * P:(tt + 1) * P],
                         env["wkv_sb"][:, d, :],
                         start=(d == 0), stop=(d == DT - 1))
    kvsb = tpool.tile([P, 2 * DH], md, tag="kvsb", bufs=3)
    nc.scalar.copy(kvsb[:], kv_ps[:])
    rr_kv = _stat_pair(nc, env, [kvsb[:, 0:DH], kvsb[:, DH:2 * DH]], 1)
    pj["kvsb"] = kvsb
    pj["rr_kv"] = rr_kv


def _emit_proj_rope(nc, tc, env, tt, pj, part):
    """part 0: q heads norm+rope; part 1: k norm+rope and v norm.  rope+norm
    in ~5 DVE ops per head:
      u  = (src * rr) * c           (c = cos table)
      v  = (rot_half(src) * rr) * s   (signs folded into s; 2 half-ops)
      qr = u + v
    Head-major transposes run on the DMA engines (XBAR 128-block transpose),
    keeping the PE free."""
    f32, md = env["f32"], env["md"]
    Alu = env["Alu"]
    kT_sb, v_sb = env["kT_sb"], env["v_sb"]
    tpool = env["tpool"]

    hd = DH // 2
    compact = env["compact_rope"]
    whichs = (0, 1) if part == 0 else (2,)
    psum = env["psum"]
    ident = env["ident"]
    qr2 = tpool.tile([P, len(whichs) * DH], md,
                     tag="qr2" if part == 0 else "kr2", bufs=3)
    for slot, which in enumerate(whichs):
        if which < NH:
            src = pj["qsb"][:, which * DH:(which + 1) * DH]
            rr = pj["rr_q"][:, which:which + 1]
        else:
            src = pj["kvsb"][:, 0:DH]
            rr = pj["rr_kv"][:, 0:1]
        if compact:
            ch = env["rope_sb"][:, tt, 0:hd]
            sn = env["rope_sb"][:, tt, hd:2 * hd]
            sp = env["rope_sb"][:, tt, 2 * hd:3 * hd]
        else:
            if which < NH:
                cw = env["cqw_sb"][:, tt, :]
                sw = env["sqw_sb"][:, tt, :]
            else:
                cw = env["ckw_sb"][:, tt, :]
                sw = env["skw_sb"][:, tt, :]
        u = tpool.tile([P, DH], md, tag="qa", bufs=3)
        if compact:
            nc.vector.scalar_tensor_tensor(u[:, 0:hd], src[:, 0:hd], rr, ch,
                                           op0=Alu.mult, op1=Alu.mult)
            nc.vector.scalar_tensor_tensor(u[:, hd:DH], src[:, hd:DH], rr, ch,
                                           op0=Alu.mult, op1=Alu.mult)
        else:
            nc.vector.scalar_tensor_tensor(u[:], src, rr, cw,
                                           op0=Alu.mult, op1=Alu.mult)
        v = tpool.tile([P, DH], md, tag="t1", bufs=3)
        nc.vector.scalar_tensor_tensor(v[:, 0:hd], src[:, hd:DH], rr,
                                       sn if compact else sw[:, 0:hd],
                                       op0=Alu.mult, op1=Alu.mult)
        nc.vector.scalar_tensor_tensor(v[:, hd:DH], src[:, 0:hd], rr,
                                       sp if compact else sw[:, hd:DH],
                                       op0=Alu.mult, op1=Alu.mult)
        nc.vector.tensor_add(qr2[:, slot * DH:(slot + 1) * DH], u[:], v[:])
    if True:
        # PE transpose into head-major layout, then one batched DVE copy
        nblk = 2 * len(whichs)
        tp_ps = psum.tile([P, 512], md, tag="t", bufs=T_BUFS, name="tp_ps")
        for blk in range(nblk):
            nc.tensor.transpose(tp_ps[:, blk * P:(blk + 1) * P],
                                qr2[:, blk * P:(blk + 1) * P], ident[:])
        if part == 0:
            qT_t = tpool.tile([P, NH * 2, P], md, tag="qTt", bufs=5,
                              name="qT_t")
            nc.vector.tensor_copy(
                qT_t[:], tp_ps[:].rearrange("p (b q1) -> p b q1", b=4))
            env["qT_tiles"][tt] = qT_t
        else:
            nc.vector.tensor_copy(
                kT_sb[:, :, tt * P:(tt + 1) * P],
                tp_ps[:, 0:2 * P].rearrange("p (b q1) -> p b q1", b=2))
    if part == 1:
        # ---- v: rms-norm only, stays token-major (fp16 SBUF -> 4x mode)
        nc.vector.tensor_scalar_mul(v_sb[:, tt, :], pj["kvsb"][:, DH:2 * DH],
                                    pj["rr_kv"][:, 1:2])


def _emit_attn_scores(nc, tc, env, i, h):
    """Scores in 512-col PSUM chunks + per-chunk max + exp.  Returns the
    packet (pchunks, zs, W) consumed by _emit_attn_rest."""
    f32, md = env["f32"], env["md"]
    Alu, Act, X = env["Alu"], env["Act"], env["X"]
    kT_sb = env["kT_sb"]
    trif_sb = env["trif_sb"]
    ppool, spool, psum = env["ppool"], env["spool"], env["psum"]
    qT_t = env["qT_tiles"][i]

    nlive = i + 1
    W = (nlive * P + KC - 1) // KC
    mxs = spool.tile([P, 5], f32, tag="mxs", name="mxs")
    schunks = []
    for c in range(W):
        k0 = c * KC
        k1 = min((c + 1) * KC, nlive * P)
        wc = k1 - k0
        s = psum.tile([P, KC], f32, tag="s", bufs=S_BUFS, name="s")
        for dh in (0, 1):
            nc.tensor.matmul(
                s[:, 0:wc], qT_t[:, h * 2 + dh, :],
                kT_sb[:, dh, k0:k1],
                start=(dh == 0), stop=(dh == 1))
        if c == W - 1:
            # additive causal mask on the diagonal 128x128 block (DVE)
            nc.vector.tensor_tensor(s[:, wc - P:wc], s[:, wc - P:wc],
                                    trif_sb[:], op=Alu.add)
        nc.vector.tensor_reduce(mxs[:, c:c + 1], s[:, 0:wc], axis=X,
                                op=Alu.max, negate=True)
        schunks.append((s, wc))
    negm = spool.tile([P, 1], f32, tag="negm", name="negm")
    nc.vector.tensor_reduce(negm[:], mxs[:, 0:W], axis=X, op=Alu.min)

    zs = spool.tile([P, 4], f32, tag="zs", name="zs")
    pchunks = []
    for c, (s, wc) in enumerate(schunks):
        p = ppool.tile([P, KC], md, tag="p")
        nc.scalar.activation(p[:, 0:wc], s[:, 0:wc], Act.Exp,
                             bias=negm[:], accum_out=zs[:, c:c + 1])
        pchunks.append((p, wc))
    return (pchunks, zs, W)


def _emit_attn_rest(nc, tc, env, i, h, pk):
    f32, md = env["f32"], env["md"]
    Alu, X = env["Alu"], env["X"]
    ident, v_sb = env["ident"], env["v_sb"]
    ptpool, tpool, spool, psum = (env["ptpool"], env["tpool"], env["spool"],
                                  env["psum"])
    pchunks, zs, W = pk
    if h == 0:
        env["aT_tiles"][i] = tpool.tile([P, NH * 2, P], md, tag="aTt",
                                        bufs=3, name="aT_t")
    aT_t = env["aT_tiles"][i]

    nlive = i + 1
    a_ps = psum.tile([P, KC], f32, tag="ao", bufs=AO_BUFS, name="a_ps")
    # all prob transposes first, then all PV matmuls: each pt copy's DVE
    # latency hides behind the next chunk's transposes instead of stalling
    # the PV chain
    pts = []
    for (p, wc) in pchunks:
        nbl = wc // P
        trp = psum.tile([P, KC], md, tag="t", bufs=T_BUFS, name="trp")
        for j in range(nbl):
            nc.tensor.transpose(trp[:, j * P:(j + 1) * P],
                                p[:, j * P:(j + 1) * P], ident[:])
        pt = ptpool.tile([P, KC], md, tag="pt")
        nc.vector.tensor_copy(pt[:, 0:nbl * P], trp[:, 0:nbl * P])
        pts.append((pt, nbl))
    gl = 0
    for (pt, nbl) in pts:
        for j in range(nbl):
            nc.tensor.matmul(a_ps[:, 0:DH], pt[:, j * P:(j + 1) * P],
                             v_sb[:, gl, :],
                             start=(gl == 0), stop=(gl == nlive - 1))
            gl += 1

    # normalize + transpose to head-major aT
    z = spool.tile([P, 1], f32, tag="z", name="z")
    nc.vector.reduce_sum(z[:], zs[:, 0:W], axis=X)
    rz = spool.tile([P, 1], f32, tag="rz", name="rz")
    nc.vector.reciprocal(rz[:], z[:])
    if True:
        at = tpool.tile([P, DH], md, tag="at", bufs=3)
        nc.vector.tensor_scalar_mul(at[:], a_ps[:, 0:DH], rz[:])
        atp = psum.tile([P, KC], md, tag="t", bufs=T_BUFS, name="atp")
        for e in range(2):
            nc.tensor.transpose(atp[:, e * P:(e + 1) * P],
                                at[:, e * P:(e + 1) * P], ident[:])
        nc.vector.tensor_copy(
            aT_t[:, h * 2:h * 2 + 2, :],
            atp[:, 0:2 * P].rearrange("p (b q1) -> p b q1", b=2))


def _emit_op_chain(nc, tc, env, i, dc):
    f32 = env["f32"]
    wo_sb, out_d = env["wo_sb"], env["out_d"]
    out_md = env["out_md"]
    opool, psum = env["opool"], env["psum"]
    aT_t = env["aT_tiles"][i]

    OC = 256  # half-bank chains: finer-grained PE filler
    ET = NH * DH // P  # 4
    o_ps = psum.tile([P, OC], f32, tag="ao", bufs=AO_BUFS, name="o_ps")
    for e in range(ET):
        nc.tensor.matmul(
            o_ps[:], aT_t[:, e, :],
            wo_sb[:, e, dc * OC:(dc + 1) * OC],
            start=(e == 0), stop=(e == ET - 1))
    o_sb = opool.tile([P, OC], out_md, tag="o")
    nc.scalar.copy(o_sb[:], o_ps[:])
    nc.sync.dma_start(
        out_d[i * P:(i + 1) * P, dc * OC:(dc + 1) * OC], o_sb[:])


def _can_compact(inputs):
    """Compact rope path needs all-ones norm weights, batch-identical
    cos/sin, and identical cos/sin halves (true for the reference RoPE)."""
    cos = np.asarray(inputs["cos"], np.float32)
    sin = np.asarray(inputs["sin"], np.float32)
    qnw = np.asarray(inputs["q_norm_w"], np.float32)
    knw = np.asarray(inputs["k_norm_w"], np.float32)
    hd = DH // 2
    return (np.all(qnw == 1.0) and np.all(knw == 1.0)
            and all(np.array_equal(cos[0], cos[b]) for b in range(1, B))
            and all(np.array_equal(sin[0], sin[b]) for b in range(1, B))
            and np.array_equal(cos[0][:, :hd], cos[0][:, hd:])
            and np.array_equal(sin[0][:, :hd], sin[0][:, hd:]))


def _host_prep(inputs, compact_rope=None):
    """Build the 8 per-core input maps from full inputs."""
    x = np.asarray(inputs["hidden_states"], np.float32)
    cos = np.asarray(inputs["cos"], np.float32)
    sin = np.asarray(inputs["sin"], np.float32)
    wq = np.asarray(inputs["wq"], np.float32)
    wk = np.asarray(inputs["wk"], np.float32)
    wv = np.asarray(inputs["wv"], np.float32)
    wo = np.asarray(inputs["wo"], np.float32)
    qnw = np.asarray(inputs["q_norm_w"], np.float32)
    knw = np.asarray(inputs["k_norm_w"], np.float32)

    if compact_rope is None:
        compact_rope = _can_compact(inputs)
    md = _np_md()
    hd = DH // 2

    if compact_rope:
        ch = cos[0][:, 0:hd]
        sh = sin[0][:, 0:hd]
        rope3 = [np.ascontiguousarray(
            np.concatenate([ch, -sh, sh], axis=1)).astype(md)] * B
    else:
        # rope tables with norm weight and rotate-half signs folded in
        sign = np.concatenate([-np.ones(hd), np.ones(hd)]).astype(np.float32)

        def _rope_tabs(w):
            w_rot = np.concatenate([w[hd:], w[:hd]])
            cw = [np.ascontiguousarray(cos[b] * w[None, :]).astype(md)
                  for b in range(B)]
            sw = [np.ascontiguousarray(
                      sin[b] * (sign * w_rot)[None, :]).astype(md)
                  for b in range(B)]
            return cw, sw

        cqw, sqw = _rope_tabs(qnw)
        ckw, skw = _rope_tabs(knw)

    # additive lower-triangular mask for the diagonal 128x128 block (fp32)
    r = np.arange(P)[:, None]
    c = np.arange(P)[None, :]
    trimaskf = np.where(c <= r, 0.0, NEG).astype(np.float32)

    xT = [np.ascontiguousarray(x[b].T).astype(md) for b in range(B)]

    in_maps = []
    for cid in range(8):
        b = cid // 4
        j = cid % 4
        h0 = 2 * j
        g = j // 2
        wqT = np.ascontiguousarray(wq[h0 * DH:(h0 + 2) * DH, :].T).astype(md)
        wkvT = np.ascontiguousarray(
            np.concatenate([wk[g * DH:(g + 1) * DH, :],
                            wv[g * DH:(g + 1) * DH, :]], axis=0).T).astype(md)
        woT2 = np.ascontiguousarray(wo[:, h0 * DH:(h0 + 2) * DH].T).astype(md)
        def v2(a):
            return a.view(np.uint16) if a.dtype.itemsize == 2 else a
        im = {
            "xT": v2(xT[b]),
            "wqT": v2(wqT),
            "wkvT": v2(wkvT),
            "woT2": v2(woT2),
            "trimaskf": trimaskf,
            "ident": v2(np.eye(P, dtype=md)),
        }
        if compact_rope:
            im["rope3"] = v2(rope3[b])
        else:
            im["cqw"] = v2(cqw[b])
            im["sqw"] = v2(sqw[b])
            im["ckw"] = v2(ckw[b])
            im["skw"] = v2(skw[b])
        in_maps.append(im)
    return in_maps


def kernel(**inputs) -> np.ndarray:
    compact = _can_compact(inputs)
    key = ("nc", compact)
    if key not in _cache:
        _cache[key] = _build_program(compact_rope=compact)
    nc = _cache[key]
    _cache["nc"] = nc  # last-built program, for the test harness
    in_maps = _host_prep(inputs, compact_rope=compact)
    res = bass_utils.run_bass_kernel_spmd(
        nc, in_maps, core_ids=list(range(8)))
    _cache["last_result"] = res
    out = np.zeros((B, S, D), np.float32)
    for cid in range(8):
        part = res.results[cid]["out"]
        if part.dtype == np.uint16:
            part = part.view(ml_dtypes.bfloat16).astype(np.float32)
        out[cid // 4] += part
    return out



# revision 50
# speedup vs baseline: 1.0380x; 1.0022x over previous
"""TRN2 Bass kernel for nn_AttentionModel_46823733461774.

Gemma3n-style attention block: qkv projection, q/k/v RMS-norm, RoPE on q/k,
GQA causal attention (no scaling; q_norm replaces 1/sqrt(d)), output proj.

Shapes (hardcoded): B=2, S=2048, D=2048, H=8, KV=2, DH=256.

Sharding over 8 cores: core c -> batch b=c//4, q-heads {2j, 2j+1} (j=c%4),
kv-head j//2.  Each core computes the projections for its batch/heads
(token-major), norms+RoPE, causal attention for its 2 heads, and a partial
output projection attn_heads @ wo_slice^T.  Host sums the 4 partials per
batch.  cos/sin replicated.

All matmuls in fp16 (same PE throughput as bf16, 8x the mantissa accuracy);
softmax statistics and accumulations in fp32.

Single merged pipeline: projection tiles and attention tiles are emitted
interleaved (attention for token-tile i follows projection of tile i+3), so
every engine keeps independent work during the softmax dependency chains.
Scores are computed in 512-column PSUM chunks from a 4-bank ring; the causal
mask is added to the diagonal chunk on DVE; row maxes are reduced per-chunk
on DVE as each chunk's matmuls finish; exp runs per-chunk on ACT; prob
transposes + PV accumulate chunk-by-chunk; 256-wide output-proj matmul
chains are used as fine-grained PE filler inside the softmax shadows.

Input DMA is pipelined in consumption order on the SP queue (per-queue DMA
transfers serialize): first xT chunk in dt-halves + wq quarters so the
tile-0 projection starts ~10us in, then wkv, rope, remaining xT chunks, wo.

Further structure: x^T is SBUF-resident; q^T / attn^T live in small ring
buffers; rms rsqrt is computed as exp(-0.5*ln(x)) so every ACT function
(square/ln/exp/copy) lives in one activation table (no 1.3us table
reloads); when the norm weights are all-ones (the reference setup), a
single packed half-table [cos|-sin|+sin] serves q and k rope (1.5MB instead
of 4MB of DMA + SBUF), with a full-table fallback otherwise.
"""

import os
from collections import deque

import numpy as np
import ml_dtypes

import concourse.bass as bass
import concourse.mybir as mybir
import concourse.tile as tile
from concourse import bacc
from concourse import bass_utils

B, S, D = 2, 2048, 2048
H, KV, DH = 8, 2, 256
EPS = 1e-6
NEG = -30000.0   # additive causal mask (fp16-representable; exp() -> 0)
P = 128
TT = S // P      # 16 token tiles
DT = D // P      # 16 contraction tiles
NH = 2           # heads per core
KC = 512         # key chunk (scores free dim; one PSUM bank)

# matmul dtype mode: "f16" | "bf16" | "f32"
MODE = os.environ.get("KERNEL_MODE", "f16")
# repeat the body N times inside the NEFF (for wall-clock HW timing)
ITERS = int(os.environ.get("KERNEL_ITERS", "1"))

_cache = {}


def _np_md():
    if MODE == "bf16":
        return ml_dtypes.bfloat16
    if MODE == "f16":
        return np.float16
    return np.float32


def _bir_md():
    if MODE == "bf16":
        return mybir.dt.bfloat16
    if MODE == "f16":
        return mybir.dt.float16
    return mybir.dt.float32


def _build_program(compact_rope=True):
    f32 = mybir.dt.float32
    md = _bir_md()
    Alu = mybir.AluOpType
    Act = mybir.ActivationFunctionType
    X = mybir.AxisListType.X
    XY = mybir.AxisListType.XY

    nc = bacc.Bacc("TRN2", target_bir_lowering=False, debug=False, num_devices=8)

    # fp16 buffers hang at the PJRT/axon boundary -> declare 2-byte inputs
    # as uint16 and bitcast to the matmul dtype on the DRAM APs.
    io2 = mybir.dt.uint16 if mybir.dt.size(md) == 2 else md
    def _in2(name, shape):
        ap = nc.dram_tensor(name, shape, io2, kind="ExternalInput").ap()
        return ap.bitcast(md) if io2 != md else ap
    xT_d = _in2("xT", [D, S])
    wqT_d = _in2("wqT", [D, NH * DH])
    wkvT_d = _in2("wkvT", [D, 2 * DH])
    woT2_d = _in2("woT2", [NH * DH, D])
    if compact_rope:
        # norm weights are all-ones and cos/sin halves are identical, so a
        # single packed table [S, 3*hd] = [cos_half | -sin_half | +sin_half]
        # serves q and k (1.5MB instead of 4MB of DMA + SBUF).
        rope3_d = _in2("rope3", [S, 3 * (DH // 2)])
    else:
        # rope tables with the norm weight and rotate-half signs folded in:
        # cw = cos*w ; sw[d<hd] = -sin[d]*w[d+hd], sw[d>=hd] = sin[d]*w[d-hd]
        cqw_d = _in2("cqw", [S, DH])
        sqw_d = _in2("sqw", [S, DH])
        ckw_d = _in2("ckw", [S, DH])
        skw_d = _in2("skw", [S, DH])
    trimaskf_d = nc.dram_tensor("trimaskf", [P, P], f32,
                                kind="ExternalInput").ap()
    ident_d = _in2("ident", [P, P])
    # output partials in bf16 (halves the out DMA; host sums in fp32).
    # 2-byte IO declared as uint16 like the inputs (PJRT boundary quirk).
    if io2 == md:  # f32 mode
        out_d = nc.dram_tensor("out", [S, D], f32, kind="ExternalOutput").ap()
        out_md = f32
    else:
        out_d = nc.dram_tensor("out", [S, D], mybir.dt.uint16,
                               kind="ExternalOutput").ap().bitcast(
                                   mybir.dt.bfloat16)
        out_md = mybir.dt.bfloat16

    with tile.TileContext(nc) as tc:
        with (
            tc.tile_pool(name="const", bufs=1) as cpool,
            tc.tile_pool(name="resid", bufs=1) as rpool,
            tc.tile_pool(name="pbuf", bufs=8) as ppool,
            tc.tile_pool(name="ptbuf", bufs=6) as ptpool,
            tc.tile_pool(name="tmp", bufs=10) as tpool,
            tc.tile_pool(name="stat", bufs=12) as spool,
            tc.tile_pool(name="obuf", bufs=3) as opool,
            tc.tile_pool(name="psum", bufs=1, space="PSUM") as psum,
        ):
            # ---- SBUF tiles for constants / weights / x ----
            wq_sb = cpool.tile([P, DT, NH * DH], md, tag="wq")
            wkv_sb = cpool.tile([P, DT, 2 * DH], md, tag="wkv")
            wo_sb = cpool.tile([P, NH * DH // P, D], md, tag="wo")
            xT_sb = cpool.tile([P, DT, S], md, tag="xT")
            hd = DH // 2
            if compact_rope:
                rope_sb = cpool.tile([P, TT, 3 * hd], md, tag="rope3")
            else:
                cqw_sb = cpool.tile([P, TT, DH], md, tag="cqw")
                sqw_sb = cpool.tile([P, TT, DH], md, tag="sqw")
                ckw_sb = cpool.tile([P, TT, DH], md, tag="ckw")
                skw_sb = cpool.tile([P, TT, DH], md, tag="skw")
            trif_sb = cpool.tile([P, P], f32, tag="trif")
            ident = cpool.tile([P, P], md, tag="ident")
            eps_sb = cpool.tile([P, 1], f32, tag="eps")

            # ---- pipelined input DMA, in consumption order ----
            # xT in 256-token chunks (512B contiguous runs, full DMA speed);
            # weights in halves so the first proj matmuls can start early;
            # rope tables chunked alongside the x tiles they feed; wo last
            # (first consumer is the tile-0 output chain, ~4 tiles in).
            XC = 256
            NXC = S // XC

            def load_xt(ci):
                nc.sync.dma_start(
                    xT_sb[:, :, ci * XC:(ci + 1) * XC],
                    xT_d[:, ci * XC:(ci + 1) * XC].rearrange(
                        "(dt p) t -> p dt t", p=P))

            def load_rope(ci):
                if compact_rope:
                    nc.sync.dma_start(
                        rope_sb[:, 2 * ci:2 * ci + 2, :],
                        rope3_d[ci * XC:(ci + 1) * XC, :].rearrange(
                            "(tt p) d1 -> p tt d1", p=P))
                else:
                    for sb, dr in ((cqw_sb, cqw_d), (sqw_sb, sqw_d),
                                   (ckw_sb, ckw_d), (skw_sb, skw_d)):
                        nc.sync.dma_start(
                            sb[:, 2 * ci:2 * ci + 2, :],
                            dr[ci * XC:(ci + 1) * XC, :].rearrange(
                                "(tt p) d1 -> p tt d1", p=P))

            # first xT chunk in dt-halves interleaved with wq quarters, so
            # the tile-0 q-projection chain starts as soon as the first
            # 0.5MB pieces land instead of after 2.5MB
            QDT = DT // 4
            def load_wq_q(qi):
                nc.sync.dma_start(
                    wq_sb[:, qi * QDT:(qi + 1) * QDT, :],
                    wqT_d[qi * D // 4:(qi + 1) * D // 4, :].rearrange(
                        "(dt p) e -> p dt e", p=P))

            nc.sync.dma_start(
                xT_sb[:, 0:DT // 2, 0:XC],
                xT_d[0:D // 2, 0:XC].rearrange("(dt p) t -> p dt t", p=P))
            load_wq_q(0)
            load_wq_q(1)
            nc.sync.dma_start(
                xT_sb[:, DT // 2:DT, 0:XC],
                xT_d[D // 2:D, 0:XC].rearrange("(dt p) t -> p dt t", p=P))
            def load_wkv_q(qi):
                nc.sync.dma_start(
                    wkv_sb[:, qi * QDT:(qi + 1) * QDT, :],
                    wkvT_d[qi * D // 4:(qi + 1) * D // 4, :].rearrange(
                        "(dt p) e -> p dt e", p=P))

            load_wq_q(2)
            load_wkv_q(0)
            load_wq_q(3)
            for qi in range(1, 4):
                load_wkv_q(qi)
            nc.sync.dma_start(trif_sb[:], trimaskf_d)
            nc.sync.dma_start(ident[:], ident_d)
            nc.gpsimd.memset(eps_sb[:], EPS)
            load_rope(0)
            for ci in range(1, 4):
                load_xt(ci)
                load_rope(ci)
            nc.sync.dma_start(
                wo_sb[:], woT2_d.rearrange("(et p) d1 -> p et d1", p=P))
            for ci in range(4, NXC):
                load_xt(ci)
                load_rope(ci)

            # ---- persistent activations ----
            kT_sb = rpool.tile([P, 2, S], md, tag="kT")
            v_sb = rpool.tile([P, TT, DH], md, tag="v")        # token-major

            env = dict(
                f32=f32, md=md, Alu=Alu, Act=Act, X=X, XY=XY,
                wq_sb=wq_sb, wkv_sb=wkv_sb, wo_sb=wo_sb, trif_sb=trif_sb,
                ident=ident, eps_sb=eps_sb, kT_sb=kT_sb,
                v_sb=v_sb, xT_sb=xT_sb, out_d=out_d, out_md=out_md,
                ppool=ppool, ptpool=ptpool, tpool=tpool,
                spool=spool, opool=opool, psum=psum,
                qT_tiles={}, aT_tiles={},
            )
            if compact_rope:
                env["rope_sb"] = rope_sb
            else:
                env.update(cqw_sb=cqw_sb, sqw_sb=sqw_sb,
                           ckw_sb=ckw_sb, skw_sb=skw_sb)
            env["compact_rope"] = compact_rope
            import contextlib
            unroll = int(os.environ.get("KERNEL_UNROLL", "1"))
            loop_ctx = (tc.For_i(0, ITERS, 1) if ITERS > 1
                        else contextlib.nullcontext())
            with loop_ctx:
                for _ in range(unroll):
                    _emit_body(nc, tc, env)

    # Activation-table pre-placement: the stock pass greedily maps each
    # activation to the FIRST act_info.json set containing its function,
    # which ping-pongs between the exp table and the ln table (1.3us reload
    # each).  Pre-place loads with our functions masked out of every set
    # before natural_log_exp_and_others, so everything first-matches that
    # one set (it contains exp+ln+square+copy+identity) and a single load
    # suffices.  Set ids keep their original act_info.json indices.
    from concourse.hw_specs import get_activation_tables
    import bass_rust as _br
    Act = mybir.ActivationFunctionType
    tables = list(get_activation_tables(nc.m.arch).items())
    target = next(idx for idx, (n, s) in enumerate(tables)
                  if n == "natural_log_exp_and_others")
    ours = {Act.Square, Act.Ln, Act.Exp, Act.Copy, Act.Identity}
    tables = [(n, (s - ours) if idx < target else s)
              for idx, (n, s) in enumerate(tables)]
    _br.insert_act_table_loads(nc, tables)

    nc.compile()
    return nc


# PSUM bank budget (8 banks): score-chunk ring "s" x4, transpose landing
# "t" x2, PV-accumulator / out-proj chain ring "ao" x2.
S_BUFS = 4
T_BUFS = 2
AO_BUFS = 2
PRE = 3  # attention for tile i is emitted after projection of tile i+PRE


def _emit_body(nc, tc, env):
    fillers = deque()

    def pop_fill(n):
        for _ in range(min(n, len(fillers))):
            fillers.popleft()()

    for t in range(TT + PRE):
        i = t - PRE
        pj = _emit_proj_q(nc, tc, env, t) if t < TT else None
        if pj is not None:
            _emit_proj_kv(nc, tc, env, t, pj)
        if 0 <= i < TT:
            # i-scaled filler: bank surplus chains early, spend them on the
            # longer softmax shadows of late (wide-W) tiles
            npop = 3 if i < 8 else (4 if i < 13 else 6)
            pk = _emit_attn_scores(nc, tc, env, i, 0)
            pop_fill(npop)
            _emit_attn_rest(nc, tc, env, i, 0, pk)
            if pj is not None:
                _emit_proj_rope(nc, tc, env, t, pj, part=0)
            pk = _emit_attn_scores(nc, tc, env, i, 1)
            pop_fill(npop)
            _emit_attn_rest(nc, tc, env, i, 1, pk)
            if pj is not None:
                _emit_proj_rope(nc, tc, env, t, pj, part=1)
            for dc in range(D // 256):
                fillers.append(
                    lambda i=i, dc=dc: _emit_op_chain(nc, tc, env, i, dc))
        elif pj is not None:
            _emit_proj_rope(nc, tc, env, t, pj, part=0)
            _emit_proj_rope(nc, tc, env, t, pj, part=1)
    pop_fill(len(fillers))


def _stat_pair(nc, env, srcs, pair):
    """rr = exp(-0.5*ln(mean(x^2)+eps)) for a pair of DH-wide sources.
    Ln+Exp share an ACT function table with Square/Copy: no table reloads."""
    f32, Act = env["f32"], env["Act"]
    tpool, spool = env["tpool"], env["spool"]
    ss2 = spool.tile([P, 2], f32, tag=f"ss{pair}", name="ss2")
    for j in (0, 1):
        sq = tpool.tile([P, DH], f32, tag="sq", bufs=2)
        nc.scalar.activation(sq[:], srcs[j], Act.Square,
                             accum_out=ss2[:, j:j + 1])
    lg2 = spool.tile([P, 2], f32, tag=f"lg{pair}", name="lg2")
    nc.scalar.activation(lg2[:], ss2[:], Act.Ln,
                         bias=env["eps_sb"][:], scale=1.0 / DH)
    rr2 = spool.tile([P, 2], f32, tag=f"rr{pair}", name="rr2")
    nc.scalar.activation(rr2[:], lg2[:], Act.Exp, scale=-0.5)
    return rr2


def _emit_proj_q(nc, tc, env, tt):
    """q projection matmuls for token tile tt + PSUM->SBUF evacuation + rms
    statistics.  The kv half is emitted separately (inside the h0 softmax
    shadow) via _emit_proj_kv."""
    f32, md = env["f32"], env["md"]
    xT_sb = env["xT_sb"]
    tpool, psum = env["tpool"], env["psum"]

    q_ps = psum.tile([P, NH * DH], f32, tag="s", bufs=S_BUFS, name="q_ps")
    for d in range(DT):
        nc.tensor.matmul(q_ps[:], xT_sb[:, d, tt * P:(tt + 1) * P],
                         env["wq_sb"][:, d, :],
                         start=(d == 0), stop=(d == DT - 1))
    qsb = tpool.tile([P, NH * DH], md, tag="qsb", bufs=3)
    nc.scalar.copy(qsb[:], q_ps[:])
    rr_q = _stat_pair(nc, env, [qsb[:, 0:DH], qsb[:, DH:2 * DH]], 0)
    return dict(qsb=qsb, rr_q=rr_q)


def _emit_proj_kv(nc, tc, env, tt, pj):
    f32, md = env["f32"], env["md"]
    xT_sb = env["xT_sb"]
    tpool, psum = env["tpool"], env["psum"]

    kv_ps = psum.tile([P, 2 * DH], f32, tag="s", bufs=S_BUFS, name="kv_ps")
    for d in range(DT):
        nc.tensor.matmul(kv_ps[:], xT_sb[:, d, tt * P:(tt + 1) * P],
                         env["wkv_sb"][:, d, :],
                         start=(d == 0), stop=(d == DT - 1))
    kvsb = tpool.tile([P, 2 * DH], md, tag="kvsb", bufs=3)
    nc.scalar.copy(kvsb[:], kv_ps[:])
    rr_kv = _stat_pair(nc, env, [kvsb[:, 0:DH], kvsb[:, DH:2 * DH]], 1)
    pj["kvsb"] = kvsb
    pj["rr_kv"] = rr_kv


def _emit_proj_rope(nc, tc, env, tt, pj, part):
    """part 0: q heads norm+rope; part 1: k norm+rope and v norm.  rope+norm
    in ~5 DVE ops per head:
      u  = (src * rr) * c           (c = cos table)
      v  = (rot_half(src) * rr) * s   (signs folded into s; 2 half-ops)
      qr = u + v
    Head-major transposes run on the DMA engines (XBAR 128-block transpose),
    keeping the PE free."""
    f32, md = env["f32"], env["md"]
    Alu = env["Alu"]
    kT_sb, v_sb = env["kT_sb"], env["v_sb"]
    tpool = env["tpool"]

    hd = DH // 2
    compact = env["compact_rope"]
    whichs = (0, 1) if part == 0 else (2,)
    psum = env["psum"]
    ident = env["ident"]
    qr2 = tpool.tile([P, len(whichs) * DH], md,
                     tag="qr2" if part == 0 else "kr2", bufs=3)
    for slot, which in enumerate(whichs):
        if which < NH:
            src = pj["qsb"][:, which * DH:(which + 1) * DH]
            rr = pj["rr_q"][:, which:which + 1]
        else:
            src = pj["kvsb"][:, 0:DH]
            rr = pj["rr_kv"][:, 0:1]
        if compact:
            ch = env["rope_sb"][:, tt, 0:hd]
            sn = env["rope_sb"][:, tt, hd:2 * hd]
            sp = env["rope_sb"][:, tt, 2 * hd:3 * hd]
        else:
            if which < NH:
                cw = env["cqw_sb"][:, tt, :]
                sw = env["sqw_sb"][:, tt, :]
            else:
                cw = env["ckw_sb"][:, tt, :]
                sw = env["skw_sb"][:, tt, :]
        u = tpool.tile([P, DH], md, tag="qa", bufs=3)
        if compact:
            nc.vector.scalar_tensor_tensor(u[:, 0:hd], src[:, 0:hd], rr, ch,
                                           op0=Alu.mult, op1=Alu.mult)
            nc.vector.scalar_tensor_tensor(u[:, hd:DH], src[:, hd:DH], rr, ch,
                                           op0=Alu.mult, op1=Alu.mult)
        else:
            nc.vector.scalar_tensor_tensor(u[:], src, rr, cw,
                                           op0=Alu.mult, op1=Alu.mult)
        v = tpool.tile([P, DH], md, tag="t1", bufs=3)
        nc.vector.scalar_tensor_tensor(v[:, 0:hd], src[:, hd:DH], rr,
                                       sn if compact else sw[:, 0:hd],
                                       op0=Alu.mult, op1=Alu.mult)
        nc.vector.scalar_tensor_tensor(v[:, hd:DH], src[:, 0:hd], rr,
                                       sp if compact else sw[:, hd:DH],
                                       op0=Alu.mult, op1=Alu.mult)
        nc.vector.tensor_add(qr2[:, slot * DH:(slot + 1) * DH], u[:], v[:])
    if True:
        # PE transpose into head-major layout, then one batched DVE copy
        nblk = 2 * len(whichs)
        tp_ps = psum.tile([P, 512], md, tag="t", bufs=T_BUFS, name="tp_ps")
        for blk in range(nblk):
            nc.tensor.transpose(tp_ps[:, blk * P:(blk + 1) * P],
                                qr2[:, blk * P:(blk + 1) * P], ident[:])
        if part == 0:
            qT_t = tpool.tile([P, NH * 2, P], md, tag="qTt", bufs=5,
                              name="qT_t")
            nc.vector.tensor_copy(
                qT_t[:], tp_ps[:].rearrange("p (b q1) -> p b q1", b=4))
            env["qT_tiles"][tt] = qT_t
        else:
            nc.vector.tensor_copy(
                kT_sb[:, :, tt * P:(tt + 1) * P],
                tp_ps[:, 0:2 * P].rearrange("p (b q1) -> p b q1", b=2))
    if part == 1:
        # ---- v: rms-norm only, stays token-major (fp16 SBUF -> 4x mode)
        nc.vector.tensor_scalar_mul(v_sb[:, tt, :], pj["kvsb"][:, DH:2 * DH],
                                    pj["rr_kv"][:, 1:2])


def _emit_attn_scores(nc, tc, env, i, h):
    """Scores in 512-col PSUM chunks + per-chunk max + exp.  Returns the
    packet (pchunks, zs, W) consumed by _emit_attn_rest."""
    f32, md = env["f32"], env["md"]
    Alu, Act, X = env["Alu"], env["Act"], env["X"]
    kT_sb = env["kT_sb"]
    trif_sb = env["trif_sb"]
    ppool, spool, psum = env["ppool"], env["spool"], env["psum"]
    qT_t = env["qT_tiles"][i]

    nlive = i + 1
    W = (nlive * P + KC - 1) // KC
    mxs = spool.tile([P, 5], f32, tag="mxs", name="mxs")
    schunks = []
    for c in range(W):
        k0 = c * KC
        k1 = min((c + 1) * KC, nlive * P)
        wc = k1 - k0
        s = psum.tile([P, KC], f32, tag="s", bufs=S_BUFS, name="s")
        for dh in (0, 1):
            nc.tensor.matmul(
                s[:, 0:wc], qT_t[:, h * 2 + dh, :],
                kT_sb[:, dh, k0:k1],
                start=(dh == 0), stop=(dh == 1))
        if c == W - 1:
            # additive causal mask on the diagonal 128x128 block (DVE)
            nc.vector.tensor_tensor(s[:, wc - P:wc], s[:, wc - P:wc],
                                    trif_sb[:], op=Alu.add)
        nc.vector.tensor_reduce(mxs[:, c:c + 1], s[:, 0:wc], axis=X,
                                op=Alu.max, negate=True)
        schunks.append((s, wc))
    negm = spool.tile([P, 1], f32, tag="negm", name="negm")
    nc.vector.tensor_reduce(negm[:], mxs[:, 0:W], axis=X, op=Alu.min)

    zs = spool.tile([P, 4], f32, tag="zs", name="zs")
    pchunks = []
    for c, (s, wc) in enumerate(schunks):
        p = ppool.tile([P, KC], md, tag="p")
        nc.scalar.activation(p[:, 0:wc], s[:, 0:wc], Act.Exp,
                             bias=negm[:], accum_out=zs[:, c:c + 1])
        pchunks.append((p, wc))
    return (pchunks, zs, W)


def _emit_attn_rest(nc, tc, env, i, h, pk):
    f32, md = env["f32"], env["md"]
    Alu, X = env["Alu"], env["X"]
    ident, v_sb = env["ident"], env["v_sb"]
    ptpool, tpool, spool, psum = (env["ptpool"], env["tpool"], env["spool"],
                                  env["psum"])
    pchunks, zs, W = pk
    if h == 0:
        env["aT_tiles"][i] = tpool.tile([P, NH * 2, P], md, tag="aTt",
                                        bufs=3, name="aT_t")
    aT_t = env["aT_tiles"][i]

    nlive = i + 1
    a_ps = psum.tile([P, KC], f32, tag="ao", bufs=AO_BUFS, name="a_ps")
    # all prob transposes first, then all PV matmuls: each pt copy's DVE
    # latency hides behind the next chunk's transposes instead of stalling
    # the PV chain
    pts = []
    for (p, wc) in pchunks:
        nbl = wc // P
        trp = psum.tile([P, KC], md, tag="t", bufs=T_BUFS, name="trp")
        for j in range(nbl):
            nc.tensor.transpose(trp[:, j * P:(j + 1) * P],
                                p[:, j * P:(j + 1) * P], ident[:])
        pt = ptpool.tile([P, KC], md, tag="pt")
        nc.vector.tensor_copy(pt[:, 0:nbl * P], trp[:, 0:nbl * P])
        pts.append((pt, nbl))
    gl = 0
    for (pt, nbl) in pts:
        for j in range(nbl):
            nc.tensor.matmul(a_ps[:, 0:DH], pt[:, j * P:(j + 1) * P],
                             v_sb[:, gl, :],
                             start=(gl == 0), stop=(gl == nlive - 1))
            gl += 1

    # normalize + transpose to head-major aT
    z = spool.tile([P, 1], f32, tag="z", name="z")
    nc.vector.reduce_sum(z[:], zs[:, 0:W], axis=X)
    rz = spool.tile([P, 1], f32, tag="rz", name="rz")
    nc.vector.reciprocal(rz[:], z[:])
    if True:
        at = tpool.tile([P, DH], md, tag="at", bufs=3)
        nc.vector.tensor_scalar_mul(at[:], a_ps[:, 0:DH], rz[:])
        atp = psum.tile([P, KC], md, tag="t", bufs=T_BUFS, name="atp")
        for e in range(2):
            nc.tensor.transpose(atp[:, e * P:(e + 1) * P],
                                at[:, e * P:(e + 1) * P], ident[:])
        nc.vector.tensor_copy(
            aT_t[:, h * 2:h * 2 + 2, :],
            atp[:, 0:2 * P].rearrange("p (b q1) -> p b q1", b=2))


def _emit_op_chain(nc, tc, env, i, dc):
    f32 = env["f32"]
    wo_sb, out_d = env["wo_sb"], env["out_d"]
    out_md = env["out_md"]
    opool, psum = env["opool"], env["psum"]
    aT_t = env["aT_tiles"][i]

    OC = 256  # half-bank chains: finer-grained PE filler
    ET = NH * DH // P  # 4
    o_ps = psum.tile([P, OC], f32, tag="ao", bufs=AO_BUFS, name="o_ps")
    for e in range(ET):
        nc.tensor.matmul(
            o_ps[:], aT_t[:, e, :],
            wo_sb[:, e, dc * OC:(dc + 1) * OC],
            start=(e == 0), stop=(e == ET - 1))
    o_sb = opool.tile([P, OC], out_md, tag="o")
    nc.scalar.copy(o_sb[:], o_ps[:])
    nc.sync.dma_start(
        out_d[i * P:(i + 1) * P, dc * OC:(dc + 1) * OC], o_sb[:])


def _can_compact(inputs):
    """Compact rope path needs all-ones norm weights, batch-identical
    cos/sin, and identical cos/sin halves (true for the reference RoPE)."""
    cos = np.asarray(inputs["cos"], np.float32)
    sin = np.asarray(inputs["sin"], np.float32)
    qnw = np.asarray(inputs["q_norm_w"], np.float32)
    knw = np.asarray(inputs["k_norm_w"], np.float32)
    hd = DH // 2
    return (np.all(qnw == 1.0) and np.all(knw == 1.0)
            and all(np.array_equal(cos[0], cos[b]) for b in range(1, B))
            and all(np.array_equal(sin[0], sin[b]) for b in range(1, B))
            and np.array_equal(cos[0][:, :hd], cos[0][:, hd:])
            and np.array_equal(sin[0][:, :hd], sin[0][:, hd:]))


def _host_prep(inputs, compact_rope=None):
    """Build the 8 per-core input maps from full inputs."""
    x = np.asarray(inputs["hidden_states"], np.float32)
    cos = np.asarray(inputs["cos"], np.float32)
    sin = np.asarray(inputs["sin"], np.float32)
    wq = np.asarray(inputs["wq"], np.float32)
    wk = np.asarray(inputs["wk"], np.float32)
    wv = np.asarray(inputs["wv"], np.float32)
    wo = np.asarray(inputs["wo"], np.float32)
    qnw = np.asarray(inputs["q_norm_w"], np.float32)
    knw = np.asarray(inputs["k_norm_w"], np.float32)

    if compact_rope is None:
        compact_rope = _can_compact(inputs)
    md = _np_md()
    hd = DH // 2

    if compact_rope:
        ch = cos[0][:, 0:hd]
        sh = sin[0][:, 0:hd]
        rope3 = [np.ascontiguousarray(
            np.concatenate([ch, -sh, sh], axis=1)).astype(md)] * B
    else:
        # rope tables with norm weight and rotate-half signs folded in
        sign = np.concatenate([-np.ones(hd), np.ones(hd)]).astype(np.float32)

        def _rope_tabs(w):
            w_rot = np.concatenate([w[hd:], w[:hd]])
            cw = [np.ascontiguousarray(cos[b] * w[None, :]).astype(md)
                  for b in range(B)]
            sw = [np.ascontiguousarray(
                      sin[b] * (sign * w_rot)[None, :]).astype(md)
                  for b in range(B)]
            return cw, sw

        cqw, sqw = _rope_tabs(qnw)
        ckw, skw = _rope_tabs(knw)

    # additive lower-triangular mask for the diagonal 128x128 block (fp32)
    r = np.arange(P)[:, None]
    c = np.arange(P)[None, :]
    trimaskf = np.where(c <= r, 0.0, NEG).astype(np.float32)

    xT = [np.ascontiguousarray(x[b].T).astype(md) for b in range(B)]

    in_maps = []
    for cid in range(8):
        b = cid // 4
        j = cid % 4
        h0 = 2 * j
        g = j // 2
        wqT = np.ascontiguousarray(wq[h0 * DH:(h0 + 2) * DH, :].T).astype(md)
        wkvT = np.ascontiguousarray(
            np.concatenate([wk[g * DH:(g + 1) * DH, :],
                            wv[g * DH:(g + 1) * DH, :]], axis=0).T).astype(md)
        woT2 = np.ascontiguousarray(wo[:, h0 * DH:(h0 + 2) * DH].T).astype(md)
        def v2(a):
            return a.view(np.uint16) if a.dtype.itemsize == 2 else a
        im = {
            "xT": v2(xT[b]),
            "wqT": v2(wqT),
            "wkvT": v2(wkvT),
            "woT2": v2(woT2),
            "trimaskf": trimaskf,
            "ident": v2(np.eye(P, dtype=md)),
        }
        if compact_rope:
            im["rope3"] = v2(rope3[b])
        else:
            im["cqw"] = v2(cqw[b])
            im["sqw"] = v2(sqw[b])
            im["ckw"] = v2(ckw[b])
            im["skw"] = v2(skw[b])
        in_maps.append(im)
    return in_maps


def kernel(**inputs) -> np.ndarray:
    compact = _can_compact(inputs)
    key = ("nc", compact)
    if key not in _cache:
        _cache[key] = _build_program(compact_rope=compact)
    nc = _cache[key]
    _cache["nc"] = nc  # last-built program, for the test harness
    in_maps = _host_prep(inputs, compact_rope=compact)
    res = bass_utils.run_bass_kernel_spmd(
        nc, in_maps, core_ids=list(range(8)))
    _cache["last_result"] = res
    out = np.zeros((B, S, D), np.float32)
    for cid in range(8):
        part = res.results[cid]["out"]
        if part.dtype == np.uint16:
            part = part.view(ml_dtypes.bfloat16).astype(np.float32)
        out[cid // 4] += part
    return out

